# revision 108
# baseline (speedup 1.0000x reference)
"""Trainium2 Bass kernel for nn_Net_60052232733176 (gnn_message_passing).

Strategy (graph-data parallel, 8 cores):
  - 50 graphs of 1000 nodes; core c handles graph slots [7c, 7c+7) (padded
    to 1024 nodes/graph).
  - Host side re-encodes each graph's edge list as a dense bf16 multiplicity
    matrix cnt[1024,1024] and ships x / x^T in bf16 (pure preprocessing).
  - On device everything is dense, mostly bf16: GAT attention logits are
    rank-1 (e_src[u]+e_dst[v]) built by PE broadcast-transposes; exp via
    two Act passes; masking by a DVE multiply with cnt; aggregation and
    per-dst softmax denominators via PE matmuls (den as one-shot per-column
    ones-matmuls per src tile — a PSUM bank only supports one OPEN
    accumulation group — summed on DVE). TopK pooling via dense rank
    counting (compare matrix + per-column ones-matmuls); readout max via
    tree reduce, mean via PE ones-matmuls; final MLP batched over graphs
    with log-softmax via Newton iterations for ln.
  - Scheduling: engine queues are strictly in-order, so the per-graph
    stages are emitted as Python generators and a sliding-window scheduler
    op-level-interleaves four independent streams (one graph's GAT+GC
    layer stages with the other graph's two topk stages, phase-offset) so
    every engine queue alternates ready work from independent chains.
  - No gather/scatter on device at all.

Self-contained: hardcodes all shapes; no file reads.
"""
import os
import numpy as np

import concourse.bass as bass
import concourse.bacc as bacc
import concourse.mybir as mybir
import concourse.tile as tile
from concourse.bass_utils import run_bass_kernel_spmd
from concourse.masks import make_identity
from concourse import bass_isa

F32 = mybir.dt.float32
BF16 = mybir.dt.bfloat16
FP16 = mybir.dt.float16
AF = mybir.ActivationFunctionType
OP = mybir.AluOpType
AX = mybir.AxisListType

P = 128
B, NPG, D, C = 50, 1000, 128, 10
NP_ = 1024            # padded nodes per graph
NT = NP_ // P         # 8 node tiles
NCORES = 8
G = 7                 # graph slots per core
K1, K2, K3 = 800, 640, 512
BIGM = 100.0          # dead-node fold added to e_src before exp
BIGS = 1.0e30         # dead-node fold for topk scores / readout max

_cache = {}


# ----------------------------------------------------------------------------
# device program
# ----------------------------------------------------------------------------

def _build_program():
    KG = int(os.environ.get("K_GRAPHS", G))
    KDBG = os.environ.get("K_DBG", "0") == "1"
    nc = bacc.Bacc(None, target_bir_lowering=False)

    # ---- DRAM tensors ----
    x_d = nc.dram_tensor("x_sh", [G, NP_, D], BF16, kind="ExternalInput")
    xT_d = nc.dram_tensor("xT_sh", [G, D, NP_], BF16, kind="ExternalInput")
    cnt_d = nc.dram_tensor("cnt_sh", [G, NP_, NP_], BF16, kind="ExternalInput")
    m0_d = nc.dram_tensor("m0", [P, NT], F32, kind="ExternalInput")

    def wparam(name, shape):
        return nc.dram_tensor(name, shape, F32, kind="ExternalInput")

    Wg_d = [wparam(f"W_g{l}", [D, D]) for l in (1, 2, 3)]
    asd_d = [wparam(f"asd_g{l}", [D, 2]) for l in (1, 2, 3)]
    bg_d = [wparam(f"b_g{l}", [D, 1]) for l in (1, 2, 3)]
    Wr_d = [wparam(f"Wr_c{l}", [D, D]) for l in (1, 2, 3)]
    br_d = [wparam(f"br_c{l}", [D, 1]) for l in (1, 2, 3)]
    Wo_d = [wparam(f"Wo_c{l}", [D, D]) for l in (1, 2, 3)]
    wp_d = {n: wparam(n, [D, 1]) for n in ("w_p20", "w_p30", "w_p11", "w_p21", "w_p31")}
    Wl1_d = wparam("W_l1", [2 * D, D])
    bl1_d = wparam("b_l1", [D, 1])
    Wl2_d = wparam("W_l2", [D, 64])
    bl2_d = wparam("b_l2", [64, 1])
    Wl3_d = wparam("W_l3", [64, C])
    bl3_d = wparam("b_l3", [C, 1])

    out_d = nc.dram_tensor("out", [G, C], F32, kind="ExternalOutput")
    dbg_d = nc.dram_tensor("dbg", [P, NP_], F32, kind="ExternalOutput") if KDBG else None

    with tile.TileContext(nc) as tc:
        import contextlib
        with contextlib.ExitStack() as ctx:
            cp = ctx.enter_context(tc.tile_pool(name="const", bufs=1))
            cbp = ctx.enter_context(tc.tile_pool(name="cntbf", bufs=3))
            Lp = ctx.enter_context(tc.tile_pool(name="Lp", bufs=6))
            Gp = ctx.enter_context(tc.tile_pool(name="Gp", bufs=6))
            stp = ctx.enter_context(tc.tile_pool(name="state", bufs=2))
            vp = ctx.enter_context(tc.tile_pool(name="vec", bufs=3))
            psA = ctx.enter_context(tc.tile_pool(name="psA", bufs=1, space="PSUM"))
            psT = ctx.enter_context(tc.tile_pool(name="psT", bufs=2, space="PSUM"))
            psR = ctx.enter_context(tc.tile_pool(name="psR", bufs=2, space="PSUM"))

            # ---- constants ----
            ident = cp.tile([P, P], F32, tag="ident")
            make_identity(nc, ident[:])
            ident_bf = cp.tile([P, P], BF16, tag="identbf")
            nc.vector.tensor_copy(out=ident_bf[:], in_=ident[:])
            ones_bf = cp.tile([P, 1], BF16, tag="onesbf")
            nc.vector.memset(ones_bf[:], 1.0)
            ones_f16 = cp.tile([P, 1], FP16, tag="onesf16")
            nc.vector.memset(ones_f16[:], 1.0)
            # PE warm: absorb gpsimd ident dep
            warm = psT.tile([P, P], F32, tag="pt")
            nc.tensor.transpose(out=warm[:], in_=ident[:], identity=ident[:])

            def load_w(dram, shape, tag, dtype=F32):
                t = cp.tile(shape, dtype, tag=tag)
                if dtype == F32:
                    nc.sync.dma_start(out=t[:], in_=dram[:])
                else:
                    tmp = cp.tile(shape, F32, tag=tag + "_st")
                    nc.sync.dma_start(out=tmp[:], in_=dram[:])
                    nc.vector.tensor_copy(out=t[:], in_=tmp[:])
                return t

            def col_to_rep(col_ap, out_tile_slice):
                """replicate a [128,1] column across partitions into [128,128]."""
                ps = psT.tile([P, P], F32, tag="pt")
                nc.tensor.transpose(out=ps[:], in_=col_ap.to_broadcast([P, P]),
                                    identity=ident[:])
                nc.vector.tensor_copy(out=out_tile_slice, in_=ps[:])

            Wg = [load_w(Wg_d[i], [D, D], f"Wg{i}", BF16) for i in range(3)]
            asd = [load_w(asd_d[i], [D, 2], f"asd{i}", BF16) for i in range(3)]
            bg_col = [load_w(bg_d[i], [D, 1], f"bg{i}") for i in range(3)]
            Wr_bf = [load_w(Wr_d[i], [D, D], f"Wr{i}", BF16) for i in range(3)]
            br_col = [load_w(br_d[i], [D, 1], f"br{i}") for i in range(3)]
            Wo_bf = [load_w(Wo_d[i], [D, D], f"Wo{i}", BF16) for i in range(3)]
            wp = {n: load_w(d, [D, 1], n) for n, d in wp_d.items()}
            Wl1a = cp.tile([D, D], F32, tag="Wl1a")
            nc.sync.dma_start(out=Wl1a[:], in_=Wl1_d[0:D, :])
            Wl1b = cp.tile([D, D], F32, tag="Wl1b")
            nc.sync.dma_start(out=Wl1b[:], in_=Wl1_d[D:2 * D, :])
            bl1 = load_w(bl1_d, [D, 1], "bl1")
            Wl2 = load_w(Wl2_d, [D, 64], "Wl2")
            bl2 = load_w(bl2_d, [64, 1], "bl2")
            Wl3 = load_w(Wl3_d, [64, C], "Wl3")
            bl3 = load_w(bl3_d, [C, 1], "bl3")
            m0 = load_w(m0_d, [P, NT], "m0")

            bg_rep = []
            for i in range(3):
                t = cp.tile([P, P], F32, tag=f"bgrep{i}")
                col_to_rep(bg_col[i][:, 0:1], t[:])
                bg_rep.append(t)
            wrep = {}
            for n in wp:
                tf = cp.tile([P, P], F32, tag=f"repf_{n}")
                col_to_rep(wp[n][:, 0:1], tf[:])
                t = cp.tile([P, P], BF16, tag=f"rep_{n}")
                nc.vector.tensor_copy(out=t[:], in_=tf[:])
                wrep[n] = t

            # per-graph readout accumulators
            gacc0 = []
            gacc1 = []
            for g in range(G):
                ga = cp.tile([P, 1], F32, tag=f"gacc0_{g}")
                gb = cp.tile([P, 1], F32, tag=f"gacc1_{g}")
                gacc0.append(ga)
                gacc1.append(gb)
            for g in range(G):
                nc.vector.memset(gacc0[g][:], 0.0)
                nc.vector.memset(gacc1[g][:], 0.0)

            CH = [slice(c * P, (c + 1) * P) for c in range(NT)]
            _dbg_hook = []
            pools_gat = [("w_p20", K1), ("w_p20", K2), ("w_p30", K3)]
            pools_gc = [("w_p11", K1), ("w_p21", K2), ("w_p31", K3)]

            def transpose_pack_bf(src_f32, dst_bf):
                """transpose [128,1024] f32 (node-major) -> [128,1024] bf16
                (feat-major) via two [128,512] PSUM half-passes."""
                for h in range(2):
                    pw = psT.tile([P, 512], F32, tag="pt")
                    for c in range(4):
                        nc.tensor.matmul(pw[:, c * P:(c + 1) * P],
                                         src_f32[:, CH[4 * h + c]], ident[:],
                                         is_transpose=True)
                    nc.scalar.activation(out=dst_bf[:, h * 512:(h + 1) * 512],
                                         in_=pw[:], func=AF.Copy)

            def chunk_tree_reduce(src, col_out, op):
                """src [128, 1024] node-major; col_out [128,1] = reduce."""
                t1 = vp.tile([P, 512], BF16, tag="rt1")
                nc.vector.tensor_tensor(out=t1[:], in0=src[:, :512], in1=src[:, 512:],
                                        op=op)
                nc.vector.tensor_tensor(out=t1[:, :256], in0=t1[:, :256],
                                        in1=t1[:, 256:], op=op)
                nc.vector.tensor_tensor(out=t1[:, :128], in0=t1[:, :128],
                                        in1=t1[:, 128:256], op=op)
                ps = psT.tile([P, P], BF16, tag="pt")
                nc.tensor.matmul(ps[:], t1[:, :128], ident_bf[:], is_transpose=True)
                nc.vector.tensor_reduce(out=col_out, in_=ps[:], axis=AX.X, op=op)

            def gat_layer(st, li, sfx="a"):
                """hT: feat-major bf16 [128,1024]. Sets st["ha"] (node-major
                [128,1024] f32, pre-pool). Generator: yields between op groups."""
                cnt_bf, hT, m_cur = st["cnt"], st["hT"], st["m_gat"]
                # hW node-major bf16 via two PSUM halves
                hW_bf = stp.tile([P, NP_], BF16, tag="hW" + sfx)
                for h in range(2):
                    pw = psT.tile([P, 512], F32, tag="pt")
                    for c in range(4):
                        nc.tensor.matmul(pw[:, c * P:(c + 1) * P],
                                         hT[:, CH[4 * h + c]], Wg[li][:],
                                         start=True, stop=True)
                    nc.scalar.activation(out=hW_bf[:, h * 512:(h + 1) * 512],
                                         in_=pw[:], func=AF.Copy)
                    yield
                # e vectors: [es | ed] per chunk (copies on DVE: keeps the
                # es2 -> Prelu-bias chain off the Act queue)
                est = vp.tile([P, 2 * NT], F32, tag="est" + sfx)
                for c in range(NT):
                    pe = psT.tile([P, 2], F32, tag="pt")
                    nc.tensor.matmul(pe[:], hT[:, CH[c]], asd[li][:],
                                     start=True, stop=True)
                    nc.vector.tensor_copy(out=est[:, 2 * c:2 * c + 2], in_=pe[:])
                    if c % 4 == 3:
                        yield
                est3 = est[:].rearrange("p (c two) -> p c two", two=2)
                est_e = est3[:, :, 0:1].rearrange("p c one -> p (c one)")
                est_o = est3[:, :, 1:2].rearrange("p c one -> p (c one)")
                # es2 = es + (m-1)*BIGM
                mf = vp.tile([P, NT], F32, tag="mf" + sfx)
                nc.vector.tensor_scalar(out=mf[:], in0=m_cur[:], scalar1=1.0,
                                        scalar2=BIGM, op0=OP.subtract, op1=OP.mult)
                es2 = vp.tile([P, NT], F32, tag="es2" + sfx)
                nc.vector.tensor_tensor(out=es2[:], in0=est_e, in1=mf[:], op=OP.add)
                yield
                # ed_rep [128, 1024] bf16 via two halves
                ed_rep = stp.tile([P, NP_], BF16, tag="edr" + sfx)
                for h in range(2):
                    pw = psT.tile([P, 512], F32, tag="pt")
                    for c in range(4):
                        cc = 4 * h + c
                        nc.tensor.transpose(
                            out=pw[:, c * P:(c + 1) * P],
                            in_=est[:, 2 * cc + 1:2 * cc + 2].to_broadcast([P, P]),
                            identity=ident[:])
                    nc.scalar.activation(out=ed_rep[:, h * 512:(h + 1) * 512],
                                         in_=pw[:], func=AF.Copy)
                    yield
                # self terms (early: off the critical tail)
                lself = vp.tile([P, NT], F32, tag="ls" + sfx)
                nc.vector.tensor_tensor(out=lself[:], in0=es2[:], in1=est_o,
                                        op=OP.add)
                nc.scalar.activation(out=lself[:], in_=lself[:], func=AF.Prelu,
                                     alpha=0.2)
                nc.scalar.activation(out=lself[:], in_=lself[:], func=AF.Exp)
                yield
                # L tiles: lrelu(ed_rep + es_u) -> exp -> * cnt; agg + den
                # matmuls (den: one-shot per-column matmuls per t — a PSUM bank
                # can only hold one OPEN accumulation group — summed on DVE)
                agg_ps = psA.tile([P, NP_], F32, tag="agA")
                den_sb = vp.tile([P, NT], F32, tag="dn" + sfx)
                for t in range(NT):
                    Lt = Lp.tile([P, NP_], BF16, tag="L")
                    nc.scalar.activation(out=Lt[:], in_=ed_rep[:], func=AF.Prelu,
                                         alpha=0.2, bias=es2[:, t:t + 1])
                    nc.scalar.activation(out=Lt[:], in_=Lt[:], func=AF.Exp)
                    yield
                    nc.vector.tensor_tensor(out=Lt[:], in0=Lt[:],
                                            in1=cnt_bf[:, t * NP_:(t + 1) * NP_],
                                            op=OP.mult)
                    for h in range(2):
                        nc.tensor.matmul(
                            agg_ps[:, h * 512:(h + 1) * 512],
                            hW_bf[:, CH[t]], Lt[:, h * 512:(h + 1) * 512],
                            start=(t == 0), stop=(t == NT - 1))
                    psd = psR.tile([P, NT], F32, tag="pc")
                    for c in range(NT):
                        nc.tensor.matmul(
                            psd[:, c:c + 1], Lt[:, CH[c]], ones_bf[:],
                            start=True, stop=True)
                    if t == 0:
                        nc.vector.tensor_copy(out=den_sb[:], in_=psd[:])
                    else:
                        nc.vector.tensor_tensor(out=den_sb[:], in0=den_sb[:],
                                                in1=psd[:], op=OP.add)
                    yield
                dtot = vp.tile([P, NT], F32, tag="dt" + sfx)
                nc.vector.tensor_tensor(out=dtot[:], in0=den_sb[:], in1=lself[:],
                                        op=OP.add)
                if os.environ.get("K_DBG_PICK", "") == f"stats{sfx}{li}":
                    dbst = stp.tile([P, NP_], F32, tag="dbst")
                    nc.vector.memset(dbst[:], 0.0)
                    nc.vector.tensor_copy(out=dbst[:, 0:NT], in_=dtot[:])
                    nc.vector.tensor_copy(out=dbst[:, NT:2 * NT], in_=es2[:])
                    nc.vector.tensor_copy(out=dbst[:, 2 * NT:3 * NT], in_=lself[:])
                    nc.vector.tensor_copy(out=dbst[:, 3 * NT:4 * NT], in_=est_o)
                    _dbg_hook.append(dbst)
                rd = vp.tile([P, NT], F32, tag="rd" + sfx)
                nc.vector.reciprocal(out=rd[:], in_=dtot[:])
                csel = vp.tile([P, NT], F32, tag="cs" + sfx)
                nc.vector.tensor_tensor(out=csel[:], in0=lself[:], in1=rd[:],
                                        op=OP.mult)
                yield
                # evacuate agg (feat-major) to SBUF for transpose
                outT_sb = stp.tile([P, NP_], F32, tag="oT" + sfx)
                for h in range(2):
                    nc.scalar.activation(out=outT_sb[:, h * 512:(h + 1) * 512],
                                         in_=agg_ps[:, h * 512:(h + 1) * 512],
                                         func=AF.Copy)
                    yield
                # finalize node-major: relu((aggT*rd + hW*csel + b) * m)
                h_next = stp.tile([P, NP_], BF16, tag="hn" + sfx)
                for h in range(2):
                    pw = psT.tile([P, 512], F32, tag="pt")
                    for c in range(4):
                        nc.tensor.matmul(pw[:, c * P:(c + 1) * P],
                                         outT_sb[:, CH[4 * h + c]], ident[:],
                                         is_transpose=True)
                    for c in range(4):
                        cc = 4 * h + c
                        f1 = vp.tile([P, P], F32, tag="f1" + sfx)
                        nc.vector.scalar_tensor_tensor(
                            out=f1[:], in0=hW_bf[:, CH[cc]],
                            scalar=csel[:, cc:cc + 1], in1=bg_rep[li][:],
                            op0=OP.mult, op1=OP.add)
                        f2 = vp.tile([P, P], F32, tag="f2" + sfx)
                        nc.vector.scalar_tensor_tensor(
                            out=f2[:], in0=pw[:, c * P:(c + 1) * P],
                            scalar=rd[:, cc:cc + 1], in1=f1[:],
                            op0=OP.mult, op1=OP.add)
                        nc.vector.tensor_scalar(
                            out=h_next[:, CH[cc]], in0=f2[:],
                            scalar1=m_cur[:, cc:cc + 1], scalar2=0.0,
                            op0=OP.mult, op1=OP.max)
                    yield
                st["ha"] = h_next
                if os.environ.get("K_DBG_PICK", "a2") == f"a{li}":
                    _dbg_hook.append(h_next)

            def gc_layer(st, li, sfx="b"):
                """GraphConv: relu((lin_rel(sum_src z) + lin_root(z)) * m).
                Sets st["hb"]. Generator."""
                cnt_bf, z_bf, zT, m_cur = st["cnt"], st["z_bf"], st["zT"], st["m_gc"]
                agg_ps = psA.tile([P, NP_], F32, tag="agB")
                for t in range(NT):
                    for h in range(2):
                        nc.tensor.matmul(
                            agg_ps[:, h * 512:(h + 1) * 512],
                            z_bf[:, CH[t]],
                            cnt_bf[:, t * NP_ + h * 512: t * NP_ + (h + 1) * 512],
                            start=(t == 0), stop=(t == NT - 1))
                    if t % 2 == 1:
                        yield
                aggT_bf = stp.tile([P, NP_], BF16, tag="agb" + sfx)
                for h in range(2):
                    nc.scalar.activation(out=aggT_bf[:, h * 512:(h + 1) * 512],
                                         in_=agg_ps[:, h * 512:(h + 1) * 512],
                                         func=AF.Copy)
                    yield
                outT_ps = psA.tile([P, NP_], F32, tag="agB")
                for h in range(2):
                    sl = slice(h * 512, (h + 1) * 512)
                    nc.tensor.matmul(outT_ps[:, sl], Wr_bf[li][:], aggT_bf[:, sl],
                                     start=True, stop=False)
                    nc.tensor.matmul(outT_ps[:, sl], Wo_bf[li][:], zT[:, sl],
                                     start=False, stop=True)
                    yield
                # + bias (per-feature = per-partition in feat-major)
                outT_sb = stp.tile([P, NP_], F32, tag="oT" + sfx)
                for h in range(2):
                    nc.scalar.activation(out=outT_sb[:, h * 512:(h + 1) * 512],
                                         in_=outT_ps[:, h * 512:(h + 1) * 512],
                                         func=AF.Identity, bias=br_col[li][:, 0:1])
                    yield
                h_next = stp.tile([P, NP_], BF16, tag="hn" + sfx)
                for h in range(2):
                    pw = psT.tile([P, 512], F32, tag="pt")
                    for c in range(4):
                        nc.tensor.matmul(pw[:, c * P:(c + 1) * P],
                                         outT_sb[:, CH[4 * h + c]], ident[:],
                                         is_transpose=True)
                    for c in range(4):
                        cc = 4 * h + c
                        nc.vector.tensor_scalar(
                            out=h_next[:, CH[cc]], in0=pw[:, c * P:(c + 1) * P],
                            scalar1=m_cur[:, cc:cc + 1], scalar2=0.0,
                            op0=OP.mult, op1=OP.max)
                    yield
                st["hb"] = h_next
                if os.environ.get("K_DBG_PICK", "a2") == f"b{li}":
                    _dbg_hook.append(h_next)

            def topk_readout(st, li, sfx):
                """pool branch output by top-k of scores; update st state;
                accumulate readout into gacc. Generator."""
                g = st["g"]
                if sfx == "a":
                    h_next, m_cur = st["ha"], st["m_gat"]
                    wn, k = pools_gat[li]
                else:
                    h_next, m_cur = st["hb"], st["m_gc"]
                    wn, k = pools_gc[li]
                wrep_t = wrep[wn]
                need_hT = li < 2
                # mask fold first: depends only on m_cur (ready at entry)
                mf = vp.tile([P, NT], F32, tag="mfs" + sfx)
                nc.vector.tensor_scalar(out=mf[:], in0=m_cur[:], scalar1=1.0,
                                        scalar2=BIGS, op0=OP.subtract, op1=OP.mult)
                # scores via fused mult + free-dim accumulate
                s = vp.tile([P, NT], F32, tag="s" + sfx)
                jnk = vp.tile([P, P], BF16, tag="jk" + sfx)
                for c in range(NT):
                    nc.vector.scalar_tensor_tensor(
                        out=jnk[:], in0=h_next[:, CH[c]], scalar=1.0,
                        in1=wrep_t[:], op0=OP.mult, op1=OP.mult,
                        accum_out=s[:, c:c + 1])
                    if c % 2 == 1:
                        yield
                # masked scores s' = s + (m-1)*BIGS
                sm = vp.tile([P, NT], F32, tag="sm" + sfx)
                nc.vector.tensor_tensor(out=sm[:], in0=s[:], in1=mf[:], op=OP.add)
                yield
                # s_rep [128, 1024] f32 via two halves
                srep = stp.tile([P, NP_], F32, tag="sr" + sfx, bufs=3)
                for h in range(2):
                    pw = psT.tile([P, 512], F32, tag="pt")
                    for c in range(4):
                        cc = 4 * h + c
                        nc.tensor.transpose(
                            out=pw[:, c * P:(c + 1) * P],
                            in_=sm[:, cc:cc + 1].to_broadcast([P, P]),
                            identity=ident[:])
                    nc.scalar.activation(out=srep[:, h * 512:(h + 1) * 512],
                                         in_=pw[:], func=AF.Copy)
                    yield
                # G[j, i] = s'[i] < s'[j]; rank[i] = sum_j G[j, i] (column form,
                # one-shot per-column matmuls per t, summed on DVE)
                th = vp.tile([P, NT], F32, tag="th" + sfx)
                nc.scalar.activation(out=th[:], in_=s[:], func=AF.Tanh)
                rank_sb = vp.tile([P, NT], F32, tag="rk" + sfx)
                for t in range(NT):
                    Gt = Gp.tile([P, NP_], FP16, tag="G")
                    eng = nc.vector if t % 2 == 0 else nc.gpsimd
                    eng.tensor_scalar(
                        out=Gt[:], in0=srep[:],
                        scalar1=sm[:, t:t + 1], scalar2=None, op0=OP.is_lt)
                    psr = psR.tile([P, NT], F32, tag="pc")
                    for c in range(NT):
                        nc.tensor.matmul(
                            psr[:, c:c + 1], Gt[:, CH[c]], ones_f16[:],
                            start=True, stop=True)
                    if t == 0:
                        nc.vector.tensor_copy(out=rank_sb[:], in_=psr[:])
                    else:
                        nc.vector.tensor_tensor(out=rank_sb[:], in0=rank_sb[:],
                                                in1=psr[:], op=OP.add)
                    yield
                keep = vp.tile([P, NT], F32, tag="kp" + sfx)
                nc.vector.tensor_scalar(out=keep[:], in0=rank_sb[:],
                                        scalar1=float(k),
                                        scalar2=None, op0=OP.is_lt)
                # pool scale = tanh(s) * keep ; kf = (keep-1)*BIGS
                pool = vp.tile([P, NT], F32, tag="pl" + sfx)
                nc.vector.tensor_tensor(out=pool[:], in0=th[:], in1=keep[:],
                                        op=OP.mult)
                kf = vp.tile([P, NT], F32, tag="kf" + sfx)
                nc.vector.tensor_scalar(out=kf[:], in0=keep[:], scalar1=1.0,
                                        scalar2=BIGS, op0=OP.subtract, op1=OP.mult)
                yield
                h_pool = stp.tile([P, NP_], BF16, tag="hp" + sfx)
                hm = stp.tile([P, NP_], BF16, tag="sr" + sfx, bufs=3)  # srep dead here
                for c in range(NT):
                    nc.vector.tensor_scalar(out=h_pool[:, CH[c]],
                                            in0=h_next[:, CH[c]],
                                            scalar1=pool[:, c:c + 1], scalar2=None,
                                            op0=OP.mult)
                    if c % 2 == 1:
                        yield
                for c in range(NT):
                    heng = nc.vector if c == 0 else nc.gpsimd
                    heng.tensor_scalar(out=hm[:, CH[c]], in0=h_next[:, CH[c]],
                                       scalar1=pool[:, c:c + 1],
                                       scalar2=kf[:, c:c + 1],
                                       op0=OP.mult, op1=OP.add)
                    if c % 2 == 1:
                        yield
                # hT for the next layer FIRST: it gates the next gat/gc
                # stage, while the readout below feeds only the final MLP
                hT_pool = None
                if need_hT:
                    hT_pool = stp.tile([P, NP_], BF16, tag="hT" + sfx)
                    for h in range(2):
                        pw = psT.tile([P, 512], BF16, tag="pt")
                        for c in range(4):
                            nc.tensor.matmul(pw[:, c * P:(c + 1) * P],
                                             h_pool[:, CH[4 * h + c]], ident_bf[:],
                                             is_transpose=True)
                        nc.scalar.activation(out=hT_pool[:, h * 512:(h + 1) * 512],
                                             in_=pw[:], func=AF.Copy)
                        yield
                # readout: masked max + mean/k
                mx = vp.tile([P, 1], F32, tag="mx" + sfx)
                chunk_tree_reduce(hm[:], mx[:], OP.max)
                nc.gpsimd.tensor_tensor(out=gacc0[g][:], in0=gacc0[g][:],
                                        in1=mx[:], op=OP.add)
                yield
                psm = psT.tile([P, 2], F32, tag="pt")
                for c in range(NT):
                    nc.tensor.matmul(psm[:, 0:1], h_pool[:, CH[c]], ones_bf[:],
                                     start=(c == 0), stop=(c == NT - 1))
                mn = vp.tile([P, 1], F32, tag="mn" + sfx)
                nc.vector.tensor_scalar(out=mn[:], in0=psm[:, 0:1], scalar1=1.0 / k,
                                        scalar2=None, op0=OP.mult)
                nc.gpsimd.tensor_tensor(out=gacc1[g][:], in0=gacc1[g][:],
                                        in1=mn[:], op=OP.add)
                yield
                if sfx == "a":
                    st["m_gat"] = keep
                    st["hT"] = hT_pool
                else:
                    st["m_gc"] = keep
                    st["zT"] = hT_pool
                    st["z_bf"] = h_pool

            def chain(*gens):
                for gg in gens:
                    yield from gg

            def drive(*streams):
                """round-robin the op streams until exhausted."""
                act = [iter(s) for s in streams if s is not None]
                while act:
                    for s in list(act):
                        try:
                            next(s)
                        except StopIteration:
                            act.remove(s)

            def issue_loads(g):
                xbf = stp.tile([P, NP_], BF16, tag="xbf")
                xbf3 = xbf[:].rearrange("p (c d) -> p c d", d=D)
                x_in3 = x_d[g].rearrange("(c p) d -> p c d", p=P)
                nc.sync.dma_start(out=xbf3[:, :, :], in_=x_in3[:, :, :])
                xT = stp.tile([P, NP_], BF16, tag="xT")
                nc.sync.dma_start(out=xT[:], in_=xT_d[g][:, :])
                cnt_t = cbp.tile([P, NT, NP_], BF16, tag="cnt")
                for q in range(4):
                    nc.sync.dma_start(
                        out=cnt_t[:, 2 * q:2 * q + 2, :],
                        in_=cnt_d[g].rearrange("(t p) v -> p t v", p=P)[
                            :, 2 * q:2 * q + 2, :])
                return dict(g=g, cnt=cnt_t[:].rearrange("p t v -> p (t v)"),
                            hT=xT, m_gat=m0, z_bf=xbf, zT=xT, m_gc=m0)

            def stage_gen(st, idx):
                """graph stage idx in 0..5: even = layers, odd = topks.
                Returns a list of independent op streams."""
                li = idx // 2
                if idx % 2 == 0:
                    return [gat_layer(st, li), gc_layer(st, li)]
                return [topk_readout(st, li, "a"),
                        topk_readout(st, li, "b")]

            # sliding schedule: graph g starts at step 6*(g//2) + (g%2);
            # every step pairs one graph's layer stage with the other's topk
            # stage (phase-offset op-level interleaving).
            start = {g: 3 * g for g in range(KG)}
            nsteps = max(start[g] + 6 for g in range(KG)) if KG else 0
            sts = {}
            for s in range(nsteps):
                for g in range(KG):
                    if start[g] - 2 == s or (start[g] <= 1 and s == 0 and g not in sts):
                        sts[g] = issue_loads(g)
                streams = []
                for g in range(KG):
                    idx = s - start[g]
                    if 0 <= idx < 6:
                        streams.extend(stage_gen(sts[g], idx))
                drive(*streams)

            if KDBG:
                dbg_src = None
                if _dbg_hook:
                    dbg_src = _dbg_hook[0]
                if dbg_src is not None:
                    dbf = vp.tile([P, NP_], F32, tag="dbf")
                    nc.vector.tensor_copy(out=dbf[:], in_=dbg_src[:])
                    nc.sync.dma_start(out=dbg_d[:], in_=dbf[:])
                else:
                    zz = vp.tile([P, NP_], F32, tag="zz")
                    nc.vector.memset(zz[:], 0.0)
                    nc.sync.dma_start(out=dbg_d[:], in_=zz[:])
            # ---- MLP over all graphs ----
            t1_ps = psT.tile([P, NT], F32, tag="pt")
            for g in range(G):
                nc.tensor.matmul(t1_ps[:, g:g + 1], Wl1a[:], gacc0[g][:],
                                 start=True, stop=False)
                nc.tensor.matmul(t1_ps[:, g:g + 1], Wl1b[:], gacc1[g][:],
                                 start=False, stop=True)
            t1 = vp.tile([P, G], F32, tag="t1")
            nc.vector.tensor_scalar(out=t1[:], in0=t1_ps[:, 0:G], scalar1=bl1[:, 0:1],
                                    scalar2=0.0, op0=OP.add, op1=OP.max)
            t2_ps = psT.tile([64, NT], F32, tag="pt")
            nc.tensor.matmul(t2_ps[:, 0:G], Wl2[:], t1[:], start=True, stop=True)
            t2p = vp.tile([64, G], F32, tag="t2p")
            nc.vector.tensor_scalar(out=t2p[:], in0=t2_ps[:, 0:G], scalar1=bl2[:, 0:1],
                                    scalar2=None, op0=OP.add)
            t2 = vp.tile([64, G], F32, tag="t2")
            nc.scalar.activation(out=t2[:], in_=t2p[:], func=AF.Prelu, alpha=0.01)
            t3_ps = psT.tile([C, 16], F32, tag="pt")
            nc.tensor.matmul(t3_ps[:, 0:G], Wl3[:], t2[:], start=True, stop=True)
            lg_cm = vp.tile([C, G], F32, tag="lgcm")
            nc.vector.tensor_scalar(out=lg_cm[:], in0=t3_ps[:, 0:G], scalar1=bl3[:, 0:1],
                                    scalar2=None, op0=OP.add)
            # transpose -> [G, C]
            lg_ps = psT.tile([G, 16], F32, tag="pt")
            nc.tensor.matmul(lg_ps[:, 0:C], lg_cm[:], ident[0:C, 0:C],
                             is_transpose=True)
            lg = vp.tile([G, C], F32, tag="lg")
            nc.vector.tensor_copy(out=lg[:], in_=lg_ps[:, 0:C])
            # log-sum-exp (logits are O(1))
            ex = vp.tile([G, C], F32, tag="ex")
            nc.scalar.activation(out=ex[:], in_=lg[:], func=AF.Exp)
            S = vp.tile([G, 1], F32, tag="S")
            nc.vector.tensor_reduce(out=S[:], in_=ex[:], axis=AX.X, op=OP.add)
            # ln(S) via Newton: y += S*exp(-y) - 1
            y = vp.tile([G, 1], F32, tag="y")
            nc.vector.memset(y[:], 2.3)
            for _ in range(6):
                eny = vp.tile([G, 1], F32, tag="eny")
                nc.scalar.activation(out=eny[:], in_=y[:], func=AF.Exp, scale=-1.0)
                nc.vector.tensor_tensor(out=eny[:], in0=eny[:], in1=S[:], op=OP.mult)
                nc.vector.tensor_scalar(out=eny[:], in0=eny[:], scalar1=1.0,
                                        scalar2=None, op0=OP.subtract)
                nc.vector.tensor_tensor(out=y[:], in0=y[:], in1=eny[:], op=OP.add)
            outt = vp.tile([G, C], F32, tag="outt")
            nc.vector.tensor_scalar(out=outt[:], in0=lg[:], scalar1=y[:, 0:1],
                                    scalar2=None, op0=OP.subtract)
            nc.sync.dma_start(out=out_d[:], in_=outt[:])

    nc.compile()
    return nc


# ----------------------------------------------------------------------------
# host side
# ----------------------------------------------------------------------------

def _prep_in_maps(inputs):
    import ml_dtypes
    BF = ml_dtypes.bfloat16
    x = np.ascontiguousarray(np.asarray(inputs["x"], np.float32))
    ei = np.asarray(inputs["edge_index"]).astype(np.int64)
    src, dst = ei[0], ei[1]
    gid = src // NPG
    sl, dl = src % NPG, dst % NPG

    cnt = np.zeros((B, NP_, NP_), np.int8)
    np.add.at(cnt, (gid, sl, dl), 1)
    cnt = cnt.astype(BF)

    x_pad = np.zeros((B, NP_, D), np.float32)
    x_pad[:, :NPG] = x.reshape(B, NPG, D)
    x_pad = x_pad.astype(BF)

    m0 = np.zeros((NP_,), np.float32)
    m0[:NPG] = 1.0
    m0_packed = np.ascontiguousarray(m0.reshape(NT, P).T)  # [P, NT]

    def col(v):
        return np.ascontiguousarray(np.asarray(v, np.float32).reshape(-1, 1))

    weights = {}
    for l in (1, 2, 3):
        weights[f"W_g{l}"] = np.ascontiguousarray(np.asarray(inputs[f"W_g{l}"], np.float32))
        Wg = np.asarray(inputs[f"W_g{l}"], np.float32)
        weights[f"asd_g{l}"] = np.ascontiguousarray(
            Wg @ np.stack([np.asarray(inputs[f"as_g{l}"], np.float32),
                           np.asarray(inputs[f"ad_g{l}"], np.float32)], axis=1))
        weights[f"b_g{l}"] = col(inputs[f"b_g{l}"])
        weights[f"Wr_c{l}"] = np.ascontiguousarray(np.asarray(inputs[f"Wr_c{l}"], np.float32))
        weights[f"br_c{l}"] = col(inputs[f"br_c{l}"])
        weights[f"Wo_c{l}"] = np.ascontiguousarray(np.asarray(inputs[f"Wo_c{l}"], np.float32))
    for n in ("w_p20", "w_p30", "w_p11", "w_p21", "w_p31"):
        w = np.asarray(inputs[n], np.float32)
        weights[n] = col(w / np.linalg.norm(w))
    weights["W_l1"] = np.ascontiguousarray(np.asarray(inputs["W_l1"], np.float32))
    weights["b_l1"] = col(inputs["b_l1"])
    weights["W_l2"] = np.ascontiguousarray(np.asarray(inputs["W_l2"], np.float32))
    weights["b_l2"] = col(inputs["b_l2"])
    weights["W_l3"] = np.ascontiguousarray(np.asarray(inputs["W_l3"], np.float32))
    weights["b_l3"] = col(inputs["b_l3"])

    in_maps = []
    for c in range(NCORES):
        lo = c * G
        hi = min(lo + G, B)
        xs = np.zeros((G, NP_, D), BF)
        cs = np.zeros((G, NP_, NP_), BF)
        if hi > lo:
            xs[:hi - lo] = x_pad[lo:hi]
            cs[:hi - lo] = cnt[lo:hi]
        xTs = np.ascontiguousarray(xs.transpose(0, 2, 1))
        im = {"x_sh": xs, "xT_sh": xTs, "cnt_sh": cs, "m0": m0_packed}
        im.update(weights)
        in_maps.append(im)
    return in_maps


def kernel(**inputs) -> np.ndarray:
    if "nc" not in _cache:
        _cache["nc"] = _build_program()
    nc = _cache["nc"]
    in_maps = _prep_in_maps(inputs)
    res = run_bass_kernel_spmd(nc, in_maps, list(range(NCORES)))
    out = np.zeros((B, C), np.float32)
    for c in range(NCORES):
        lo = c * G
        hi = min(lo + G, B)
        if hi > lo:
            out[lo:hi] = np.asarray(res.results[c]["out"])[:hi - lo]
    return out


# revision 109
# speedup vs baseline: 1.0001x; 1.0001x over previous
"""Trainium2 Bass kernel for nn_Net_60052232733176 (gnn_message_passing).

Strategy (graph-data parallel, 8 cores):
  - 50 graphs of 1000 nodes; core c handles graph slots [7c, 7c+7) (padded
    to 1024 nodes/graph).
  - Host side re-encodes each graph's edge list as a dense bf16 multiplicity
    matrix cnt[1024,1024] and ships x / x^T in bf16 (pure preprocessing).
  - On device everything is dense, mostly bf16: GAT attention logits are
    rank-1 (e_src[u]+e_dst[v]) built by PE broadcast-transposes; exp via
    two Act passes; masking by a DVE multiply with cnt; aggregation and
    per-dst softmax denominators via PE matmuls (den as one-shot per-column
    ones-matmuls per src tile — a PSUM bank only supports one OPEN
    accumulation group — summed on DVE). TopK pooling via dense rank
    counting (compare matrix + per-column ones-matmuls); readout max via
    tree reduce, mean via PE ones-matmuls; final MLP batched over graphs
    with log-softmax via Newton iterations for ln.
  - Scheduling: engine queues are strictly in-order, so the per-graph
    stages are emitted as Python generators and a sliding-window scheduler
    op-level-interleaves four independent streams (one graph's GAT+GC
    layer stages with the other graph's two topk stages, phase-offset) so
    every engine queue alternates ready work from independent chains.
  - No gather/scatter on device at all.

Self-contained: hardcodes all shapes; no file reads.
"""
import os
import numpy as np

import concourse.bass as bass
import concourse.bacc as bacc
import concourse.mybir as mybir
import concourse.tile as tile
from concourse.bass_utils import run_bass_kernel_spmd
from concourse.masks import make_identity
from concourse import bass_isa

F32 = mybir.dt.float32
BF16 = mybir.dt.bfloat16
FP16 = mybir.dt.float16
AF = mybir.ActivationFunctionType
OP = mybir.AluOpType
AX = mybir.AxisListType

P = 128
B, NPG, D, C = 50, 1000, 128, 10
NP_ = 1024            # padded nodes per graph
NT = NP_ // P         # 8 node tiles
NCORES = 8
G = 7                 # graph slots per core
K1, K2, K3 = 800, 640, 512
BIGM = 100.0          # dead-node fold added to e_src before exp
BIGS = 1.0e30         # dead-node fold for topk scores / readout max

_cache = {}


# ----------------------------------------------------------------------------
# device program
# ----------------------------------------------------------------------------

def _build_program():
    KG = int(os.environ.get("K_GRAPHS", G))
    KDBG = os.environ.get("K_DBG", "0") == "1"
    nc = bacc.Bacc(None, target_bir_lowering=False)

    # ---- DRAM tensors ----
    x_d = nc.dram_tensor("x_sh", [G, NP_, D], BF16, kind="ExternalInput")
    xT_d = nc.dram_tensor("xT_sh", [G, D, NP_], BF16, kind="ExternalInput")
    cnt_d = nc.dram_tensor("cnt_sh", [G, NP_, NP_], BF16, kind="ExternalInput")
    m0_d = nc.dram_tensor("m0", [P, NT], F32, kind="ExternalInput")

    def wparam(name, shape):
        return nc.dram_tensor(name, shape, F32, kind="ExternalInput")

    Wg_d = [wparam(f"W_g{l}", [D, D]) for l in (1, 2, 3)]
    asd_d = [wparam(f"asd_g{l}", [D, 2]) for l in (1, 2, 3)]
    bg_d = [wparam(f"b_g{l}", [D, 1]) for l in (1, 2, 3)]
    Wr_d = [wparam(f"Wr_c{l}", [D, D]) for l in (1, 2, 3)]
    br_d = [wparam(f"br_c{l}", [D, 1]) for l in (1, 2, 3)]
    Wo_d = [wparam(f"Wo_c{l}", [D, D]) for l in (1, 2, 3)]
    wp_d = {n: wparam(n, [D, 1]) for n in ("w_p20", "w_p30", "w_p11", "w_p21", "w_p31")}
    Wl1_d = wparam("W_l1", [2 * D, D])
    bl1_d = wparam("b_l1", [D, 1])
    Wl2_d = wparam("W_l2", [D, 64])
    bl2_d = wparam("b_l2", [64, 1])
    Wl3_d = wparam("W_l3", [64, C])
    bl3_d = wparam("b_l3", [C, 1])

    out_d = nc.dram_tensor("out", [G, C], F32, kind="ExternalOutput")
    dbg_d = nc.dram_tensor("dbg", [P, NP_], F32, kind="ExternalOutput") if KDBG else None

    with tile.TileContext(nc) as tc:
        import contextlib
        with contextlib.ExitStack() as ctx:
            cp = ctx.enter_context(tc.tile_pool(name="const", bufs=1))
            cbp = ctx.enter_context(tc.tile_pool(name="cntbf", bufs=3))
            Lp = ctx.enter_context(tc.tile_pool(name="Lp", bufs=6))
            Gp = ctx.enter_context(tc.tile_pool(name="Gp", bufs=6))
            stp = ctx.enter_context(tc.tile_pool(name="state", bufs=2))
            vp = ctx.enter_context(tc.tile_pool(name="vec", bufs=3))
            psA = ctx.enter_context(tc.tile_pool(name="psA", bufs=1, space="PSUM"))
            psT = ctx.enter_context(tc.tile_pool(name="psT", bufs=2, space="PSUM"))
            psR = ctx.enter_context(tc.tile_pool(name="psR", bufs=2, space="PSUM"))

            # ---- constants ----
            ident = cp.tile([P, P], F32, tag="ident")
            make_identity(nc, ident[:])
            ident_bf = cp.tile([P, P], BF16, tag="identbf")
            nc.vector.tensor_copy(out=ident_bf[:], in_=ident[:])
            ones_bf = cp.tile([P, 1], BF16, tag="onesbf")
            nc.vector.memset(ones_bf[:], 1.0)
            ones_f16 = cp.tile([P, 1], FP16, tag="onesf16")
            nc.vector.memset(ones_f16[:], 1.0)
            # PE warm: absorb gpsimd ident dep
            warm = psT.tile([P, P], F32, tag="pt")
            nc.tensor.transpose(out=warm[:], in_=ident[:], identity=ident[:])

            def load_w(dram, shape, tag, dtype=F32):
                t = cp.tile(shape, dtype, tag=tag)
                if dtype == F32:
                    nc.sync.dma_start(out=t[:], in_=dram[:])
                else:
                    tmp = cp.tile(shape, F32, tag=tag + "_st")
                    nc.sync.dma_start(out=tmp[:], in_=dram[:])
                    nc.vector.tensor_copy(out=t[:], in_=tmp[:])
                return t

            def col_to_rep(col_ap, out_tile_slice):
                """replicate a [128,1] column across partitions into [128,128]."""
                ps = psT.tile([P, P], F32, tag="pt")
                nc.tensor.transpose(out=ps[:], in_=col_ap.to_broadcast([P, P]),
                                    identity=ident[:])
                nc.vector.tensor_copy(out=out_tile_slice, in_=ps[:])

            Wg = [load_w(Wg_d[i], [D, D], f"Wg{i}", BF16) for i in range(3)]
            asd = [load_w(asd_d[i], [D, 2], f"asd{i}", BF16) for i in range(3)]
            bg_col = [load_w(bg_d[i], [D, 1], f"bg{i}") for i in range(3)]
            Wr_bf = [load_w(Wr_d[i], [D, D], f"Wr{i}", BF16) for i in range(3)]
            br_col = [load_w(br_d[i], [D, 1], f"br{i}") for i in range(3)]
            Wo_bf = [load_w(Wo_d[i], [D, D], f"Wo{i}", BF16) for i in range(3)]
            wp = {n: load_w(d, [D, 1], n) for n, d in wp_d.items()}
            Wl1a = cp.tile([D, D], F32, tag="Wl1a")
            nc.sync.dma_start(out=Wl1a[:], in_=Wl1_d[0:D, :])
            Wl1b = cp.tile([D, D], F32, tag="Wl1b")
            nc.sync.dma_start(out=Wl1b[:], in_=Wl1_d[D:2 * D, :])
            bl1 = load_w(bl1_d, [D, 1], "bl1")
            Wl2 = load_w(Wl2_d, [D, 64], "Wl2")
            bl2 = load_w(bl2_d, [64, 1], "bl2")
            Wl3 = load_w(Wl3_d, [64, C], "Wl3")
            bl3 = load_w(bl3_d, [C, 1], "bl3")
            m0 = load_w(m0_d, [P, NT], "m0")

            bg_rep = []
            for i in range(3):
                t = cp.tile([P, P], F32, tag=f"bgrep{i}")
                col_to_rep(bg_col[i][:, 0:1], t[:])
                bg_rep.append(t)
            wrep = {}
            for n in wp:
                tf = cp.tile([P, P], F32, tag=f"repf_{n}")
                col_to_rep(wp[n][:, 0:1], tf[:])
                t = cp.tile([P, P], BF16, tag=f"rep_{n}")
                nc.vector.tensor_copy(out=t[:], in_=tf[:])
                wrep[n] = t

            # per-graph readout accumulators
            gacc0 = []
            gacc1 = []
            for g in range(G):
                ga = cp.tile([P, 1], F32, tag=f"gacc0_{g}")
                gb = cp.tile([P, 1], F32, tag=f"gacc1_{g}")
                gacc0.append(ga)
                gacc1.append(gb)
            for g in range(G):
                nc.vector.memset(gacc0[g][:], 0.0)
                nc.vector.memset(gacc1[g][:], 0.0)

            CH = [slice(c * P, (c + 1) * P) for c in range(NT)]
            _dbg_hook = []
            pools_gat = [("w_p20", K1), ("w_p20", K2), ("w_p30", K3)]
            pools_gc = [("w_p11", K1), ("w_p21", K2), ("w_p31", K3)]

            def transpose_pack_bf(src_f32, dst_bf):
                """transpose [128,1024] f32 (node-major) -> [128,1024] bf16
                (feat-major) via two [128,512] PSUM half-passes."""
                for h in range(2):
                    pw = psT.tile([P, 512], F32, tag="pt")
                    for c in range(4):
                        nc.tensor.matmul(pw[:, c * P:(c + 1) * P],
                                         src_f32[:, CH[4 * h + c]], ident[:],
                                         is_transpose=True)
                    nc.scalar.activation(out=dst_bf[:, h * 512:(h + 1) * 512],
                                         in_=pw[:], func=AF.Copy)

            def chunk_tree_reduce(src, col_out, op):
                """src [128, 1024] node-major; col_out [128,1] = reduce."""
                t1 = vp.tile([P, 512], BF16, tag="rt1")
                nc.vector.tensor_tensor(out=t1[:], in0=src[:, :512], in1=src[:, 512:],
                                        op=op)
                nc.vector.tensor_tensor(out=t1[:, :256], in0=t1[:, :256],
                                        in1=t1[:, 256:], op=op)
                nc.vector.tensor_tensor(out=t1[:, :128], in0=t1[:, :128],
                                        in1=t1[:, 128:256], op=op)
                ps = psT.tile([P, P], BF16, tag="pt")
                nc.tensor.matmul(ps[:], t1[:, :128], ident_bf[:], is_transpose=True)
                nc.vector.tensor_reduce(out=col_out, in_=ps[:], axis=AX.X, op=op)

            def gat_layer(st, li, sfx="a"):
                """hT: feat-major bf16 [128,1024]. Sets st["ha"] (node-major
                [128,1024] f32, pre-pool). Generator: yields between op groups."""
                cnt_bf, hT, m_cur = st["cnt"], st["hT"], st["m_gat"]
                # hW node-major bf16 via two PSUM halves
                hW_bf = stp.tile([P, NP_], BF16, tag="hW" + sfx)
                for h in range(2):
                    pw = psT.tile([P, 512], F32, tag="pt")
                    for c in range(4):
                        nc.tensor.matmul(pw[:, c * P:(c + 1) * P],
                                         hT[:, CH[4 * h + c]], Wg[li][:],
                                         start=True, stop=True)
                    nc.scalar.activation(out=hW_bf[:, h * 512:(h + 1) * 512],
                                         in_=pw[:], func=AF.Copy)
                    yield
                # e vectors: [es | ed] per chunk (copies on DVE: keeps the
                # es2 -> Prelu-bias chain off the Act queue)
                est = vp.tile([P, 2 * NT], F32, tag="est" + sfx)
                for c in range(NT):
                    pe = psT.tile([P, 2], F32, tag="pt")
                    nc.tensor.matmul(pe[:], hT[:, CH[c]], asd[li][:],
                                     start=True, stop=True)
                    nc.vector.tensor_copy(out=est[:, 2 * c:2 * c + 2], in_=pe[:])
                    if c % 4 == 3:
                        yield
                est3 = est[:].rearrange("p (c two) -> p c two", two=2)
                est_e = est3[:, :, 0:1].rearrange("p c one -> p (c one)")
                est_o = est3[:, :, 1:2].rearrange("p c one -> p (c one)")
                # es2 = es + (m-1)*BIGM
                mf = vp.tile([P, NT], F32, tag="mf" + sfx)
                nc.vector.tensor_scalar(out=mf[:], in0=m_cur[:], scalar1=1.0,
                                        scalar2=BIGM, op0=OP.subtract, op1=OP.mult)
                es2 = vp.tile([P, NT], F32, tag="es2" + sfx)
                nc.vector.tensor_tensor(out=es2[:], in0=est_e, in1=mf[:], op=OP.add)
                yield
                # ed_rep [128, 1024] bf16 via two halves
                ed_rep = stp.tile([P, NP_], BF16, tag="edr" + sfx)
                for h in range(2):
                    pw = psT.tile([P, 512], F32, tag="pt")
                    for c in range(4):
                        cc = 4 * h + c
                        nc.tensor.transpose(
                            out=pw[:, c * P:(c + 1) * P],
                            in_=est[:, 2 * cc + 1:2 * cc + 2].to_broadcast([P, P]),
                            identity=ident[:])
                    nc.scalar.activation(out=ed_rep[:, h * 512:(h + 1) * 512],
                                         in_=pw[:], func=AF.Copy)
                    yield
                # self terms (early: off the critical tail)
                lself = vp.tile([P, NT], F32, tag="ls" + sfx)
                nc.vector.tensor_tensor(out=lself[:], in0=es2[:], in1=est_o,
                                        op=OP.add)
                nc.scalar.activation(out=lself[:], in_=lself[:], func=AF.Prelu,
                                     alpha=0.2)
                nc.scalar.activation(out=lself[:], in_=lself[:], func=AF.Exp)
                yield
                # L tiles: lrelu(ed_rep + es_u) -> exp -> * cnt; agg + den
                # matmuls (den: one-shot per-column matmuls per t — a PSUM bank
                # can only hold one OPEN accumulation group — summed on DVE)
                agg_ps = psA.tile([P, NP_], F32, tag="agA")
                den_sb = vp.tile([P, NT], F32, tag="dn" + sfx)
                for t in range(NT):
                    Lt = Lp.tile([P, NP_], BF16, tag="L")
                    nc.scalar.activation(out=Lt[:], in_=ed_rep[:], func=AF.Prelu,
                                         alpha=0.2, bias=es2[:, t:t + 1])
                    nc.scalar.activation(out=Lt[:], in_=Lt[:], func=AF.Exp)
                    yield
                    nc.vector.tensor_tensor(out=Lt[:], in0=Lt[:],
                                            in1=cnt_bf[:, t * NP_:(t + 1) * NP_],
                                            op=OP.mult)
                    for h in range(2):
                        nc.tensor.matmul(
                            agg_ps[:, h * 512:(h + 1) * 512],
                            hW_bf[:, CH[t]], Lt[:, h * 512:(h + 1) * 512],
                            start=(t == 0), stop=(t == NT - 1))
                    psd = psR.tile([P, NT], F32, tag="pc")
                    for c in range(NT):
                        nc.tensor.matmul(
                            psd[:, c:c + 1], Lt[:, CH[c]], ones_bf[:],
                            start=True, stop=True)
                    if t == 0:
                        nc.vector.tensor_copy(out=den_sb[:], in_=psd[:])
                    else:
                        nc.vector.tensor_tensor(out=den_sb[:], in0=den_sb[:],
                                                in1=psd[:], op=OP.add)
                    yield
                dtot = vp.tile([P, NT], F32, tag="dt" + sfx)
                nc.vector.tensor_tensor(out=dtot[:], in0=den_sb[:], in1=lself[:],
                                        op=OP.add)
                if os.environ.get("K_DBG_PICK", "") == f"stats{sfx}{li}":
                    dbst = stp.tile([P, NP_], F32, tag="dbst")
                    nc.vector.memset(dbst[:], 0.0)
                    nc.vector.tensor_copy(out=dbst[:, 0:NT], in_=dtot[:])
                    nc.vector.tensor_copy(out=dbst[:, NT:2 * NT], in_=es2[:])
                    nc.vector.tensor_copy(out=dbst[:, 2 * NT:3 * NT], in_=lself[:])
                    nc.vector.tensor_copy(out=dbst[:, 3 * NT:4 * NT], in_=est_o)
                    _dbg_hook.append(dbst)
                rd = vp.tile([P, NT], F32, tag="rd" + sfx)
                nc.vector.reciprocal(out=rd[:], in_=dtot[:])
                csel = vp.tile([P, NT], F32, tag="cs" + sfx)
                nc.vector.tensor_tensor(out=csel[:], in0=lself[:], in1=rd[:],
                                        op=OP.mult)
                yield
                # evacuate agg (feat-major) to SBUF for transpose
                outT_sb = stp.tile([P, NP_], F32, tag="oT" + sfx)
                for h in range(2):
                    nc.scalar.activation(out=outT_sb[:, h * 512:(h + 1) * 512],
                                         in_=agg_ps[:, h * 512:(h + 1) * 512],
                                         func=AF.Copy)
                    yield
                # finalize node-major: relu((aggT*rd + hW*csel + b) * m)
                h_next = stp.tile([P, NP_], BF16, tag="hn" + sfx)
                for h in range(2):
                    pw = psT.tile([P, 512], F32, tag="pt")
                    for c in range(4):
                        nc.tensor.matmul(pw[:, c * P:(c + 1) * P],
                                         outT_sb[:, CH[4 * h + c]], ident[:],
                                         is_transpose=True)
                    for c in range(4):
                        cc = 4 * h + c
                        f1 = vp.tile([P, P], F32, tag="f1" + sfx)
                        nc.vector.scalar_tensor_tensor(
                            out=f1[:], in0=hW_bf[:, CH[cc]],
                            scalar=csel[:, cc:cc + 1], in1=bg_rep[li][:],
                            op0=OP.mult, op1=OP.add)
                        f2 = vp.tile([P, P], F32, tag="f2" + sfx)
                        nc.vector.scalar_tensor_tensor(
                            out=f2[:], in0=pw[:, c * P:(c + 1) * P],
                            scalar=rd[:, cc:cc + 1], in1=f1[:],
                            op0=OP.mult, op1=OP.add)
                        nc.vector.tensor_scalar(
                            out=h_next[:, CH[cc]], in0=f2[:],
                            scalar1=m_cur[:, cc:cc + 1], scalar2=0.0,
                            op0=OP.mult, op1=OP.max)
                    yield
                st["ha"] = h_next
                if os.environ.get("K_DBG_PICK", "a2") == f"a{li}":
                    _dbg_hook.append(h_next)

            def gc_layer(st, li, sfx="b"):
                """GraphConv: relu((lin_rel(sum_src z) + lin_root(z)) * m).
                Sets st["hb"]. Generator."""
                cnt_bf, z_bf, zT, m_cur = st["cnt"], st["z_bf"], st["zT"], st["m_gc"]
                agg_ps = psA.tile([P, NP_], F32, tag="agB")
                for t in range(NT):
                    for h in range(2):
                        nc.tensor.matmul(
                            agg_ps[:, h * 512:(h + 1) * 512],
                            z_bf[:, CH[t]],
                            cnt_bf[:, t * NP_ + h * 512: t * NP_ + (h + 1) * 512],
                            start=(t == 0), stop=(t == NT - 1))
                    if t % 2 == 1:
                        yield
                aggT_bf = stp.tile([P, NP_], BF16, tag="agb" + sfx)
                for h in range(2):
                    nc.scalar.activation(out=aggT_bf[:, h * 512:(h + 1) * 512],
                                         in_=agg_ps[:, h * 512:(h + 1) * 512],
                                         func=AF.Copy)
                    yield
                outT_ps = psA.tile([P, NP_], F32, tag="agB")
                for h in range(2):
                    sl = slice(h * 512, (h + 1) * 512)
                    nc.tensor.matmul(outT_ps[:, sl], Wr_bf[li][:], aggT_bf[:, sl],
                                     start=True, stop=False)
                    nc.tensor.matmul(outT_ps[:, sl], Wo_bf[li][:], zT[:, sl],
                                     start=False, stop=True)
                    yield
                # + bias (per-feature = per-partition in feat-major)
                outT_sb = stp.tile([P, NP_], F32, tag="oT" + sfx)
                for h in range(2):
                    nc.scalar.activation(out=outT_sb[:, h * 512:(h + 1) * 512],
                                         in_=outT_ps[:, h * 512:(h + 1) * 512],
                                         func=AF.Identity, bias=br_col[li][:, 0:1])
                    yield
                h_next = stp.tile([P, NP_], BF16, tag="hn" + sfx)
                for h in range(2):
                    pw = psT.tile([P, 512], F32, tag="pt")
                    for c in range(4):
                        nc.tensor.matmul(pw[:, c * P:(c + 1) * P],
                                         outT_sb[:, CH[4 * h + c]], ident[:],
                                         is_transpose=True)
                    for c in range(4):
                        cc = 4 * h + c
                        nc.vector.tensor_scalar(
                            out=h_next[:, CH[cc]], in0=pw[:, c * P:(c + 1) * P],
                            scalar1=m_cur[:, cc:cc + 1], scalar2=0.0,
                            op0=OP.mult, op1=OP.max)
                    yield
                st["hb"] = h_next
                if os.environ.get("K_DBG_PICK", "a2") == f"b{li}":
                    _dbg_hook.append(h_next)

            def topk_readout(st, li, sfx):
                """pool branch output by top-k of scores; update st state;
                accumulate readout into gacc. Generator."""
                g = st["g"]
                if sfx == "a":
                    h_next, m_cur = st["ha"], st["m_gat"]
                    wn, k = pools_gat[li]
                else:
                    h_next, m_cur = st["hb"], st["m_gc"]
                    wn, k = pools_gc[li]
                wrep_t = wrep[wn]
                need_hT = li < 2
                # mask fold first: depends only on m_cur (ready at entry)
                mf = vp.tile([P, NT], F32, tag="mfs" + sfx)
                nc.vector.tensor_scalar(out=mf[:], in0=m_cur[:], scalar1=1.0,
                                        scalar2=BIGS, op0=OP.subtract, op1=OP.mult)
                # scores via fused mult + free-dim accumulate
                s = vp.tile([P, NT], F32, tag="s" + sfx)
                jnk = vp.tile([P, P], BF16, tag="jk" + sfx)
                for c in range(NT):
                    nc.vector.scalar_tensor_tensor(
                        out=jnk[:], in0=h_next[:, CH[c]], scalar=1.0,
                        in1=wrep_t[:], op0=OP.mult, op1=OP.mult,
                        accum_out=s[:, c:c + 1])
                    if c % 2 == 1:
                        yield
                # masked scores s' = s + (m-1)*BIGS
                sm = vp.tile([P, NT], F32, tag="sm" + sfx)
                nc.vector.tensor_tensor(out=sm[:], in0=s[:], in1=mf[:], op=OP.add)
                yield
                # s_rep [128, 1024] f32 via two halves
                srep = stp.tile([P, NP_], F32, tag="sr" + sfx, bufs=3)
                for h in range(2):
                    pw = psT.tile([P, 512], F32, tag="pt")
                    for c in range(4):
                        cc = 4 * h + c
                        nc.tensor.transpose(
                            out=pw[:, c * P:(c + 1) * P],
                            in_=sm[:, cc:cc + 1].to_broadcast([P, P]),
                            identity=ident[:])
                    nc.scalar.activation(out=srep[:, h * 512:(h + 1) * 512],
                                         in_=pw[:], func=AF.Copy)
                    yield
                # G[j, i] = s'[i] < s'[j]; rank[i] = sum_j G[j, i] (column form,
                # one-shot per-column matmuls per t, summed on DVE)
                th = vp.tile([P, NT], F32, tag="th" + sfx)
                nc.scalar.activation(out=th[:], in_=s[:], func=AF.Tanh)
                rank_sb = vp.tile([P, NT], F32, tag="rk" + sfx)
                for t in range(NT):
                    Gt = Gp.tile([P, NP_], FP16, tag="G")
                    eng = nc.vector if t % 2 == 0 else nc.gpsimd
                    eng.tensor_scalar(
                        out=Gt[:], in0=srep[:],
                        scalar1=sm[:, t:t + 1], scalar2=None, op0=OP.is_lt)
                    psr = psR.tile([P, NT], F32, tag="pc")
                    for c in range(NT):
                        nc.tensor.matmul(
                            psr[:, c:c + 1], Gt[:, CH[c]], ones_f16[:],
                            start=True, stop=True)
                    if t == 0:
                        nc.vector.tensor_copy(out=rank_sb[:], in_=psr[:])
                    else:
                        nc.vector.tensor_tensor(out=rank_sb[:], in0=rank_sb[:],
                                                in1=psr[:], op=OP.add)
                    yield
                keep = vp.tile([P, NT], F32, tag="kp" + sfx)
                nc.vector.tensor_scalar(out=keep[:], in0=rank_sb[:],
                                        scalar1=float(k),
                                        scalar2=None, op0=OP.is_lt)
                # pool scale = tanh(s) * keep ; kf = (keep-1)*BIGS
                pool = vp.tile([P, NT], F32, tag="pl" + sfx)
                nc.vector.tensor_tensor(out=pool[:], in0=th[:], in1=keep[:],
                                        op=OP.mult)
                kf = vp.tile([P, NT], F32, tag="kf" + sfx)
                nc.vector.tensor_scalar(out=kf[:], in0=keep[:], scalar1=1.0,
                                        scalar2=BIGS, op0=OP.subtract, op1=OP.mult)
                yield
                h_pool = stp.tile([P, NP_], BF16, tag="hp" + sfx)
                hm = stp.tile([P, NP_], BF16, tag="sr" + sfx, bufs=3)  # srep dead here
                for c in range(NT):
                    nc.vector.tensor_scalar(out=h_pool[:, CH[c]],
                                            in0=h_next[:, CH[c]],
                                            scalar1=pool[:, c:c + 1], scalar2=None,
                                            op0=OP.mult)
                    if c % 2 == 1:
                        yield
                for c in range(NT):
                    heng = nc.vector if c == 0 else nc.gpsimd
                    heng.tensor_scalar(out=hm[:, CH[c]], in0=h_next[:, CH[c]],
                                       scalar1=pool[:, c:c + 1],
                                       scalar2=kf[:, c:c + 1],
                                       op0=OP.mult, op1=OP.add)
                    if c % 2 == 1:
                        yield
                # hT for the next layer FIRST: it gates the next gat/gc
                # stage, while the readout below feeds only the final MLP
                hT_pool = None
                if need_hT:
                    hT_pool = stp.tile([P, NP_], BF16, tag="hT" + sfx)
                    for h in range(2):
                        pw = psT.tile([P, 512], BF16, tag="pt")
                        for c in range(4):
                            nc.tensor.matmul(pw[:, c * P:(c + 1) * P],
                                             h_pool[:, CH[4 * h + c]], ident_bf[:],
                                             is_transpose=True)
                        nc.scalar.activation(out=hT_pool[:, h * 512:(h + 1) * 512],
                                             in_=pw[:], func=AF.Copy)
                        yield
                # readout: masked max + mean/k
                mx = vp.tile([P, 1], F32, tag="mx" + sfx)
                chunk_tree_reduce(hm[:], mx[:], OP.max)
                nc.gpsimd.tensor_tensor(out=gacc0[g][:], in0=gacc0[g][:],
                                        in1=mx[:], op=OP.add)
                yield
                psm = psT.tile([P, 2], F32, tag="pt")
                for c in range(NT):
                    nc.tensor.matmul(psm[:, 0:1], h_pool[:, CH[c]], ones_bf[:],
                                     start=(c == 0), stop=(c == NT - 1))
                mn = vp.tile([P, 1], F32, tag="mn" + sfx)
                nc.vector.tensor_scalar(out=mn[:], in0=psm[:, 0:1], scalar1=1.0 / k,
                                        scalar2=None, op0=OP.mult)
                nc.gpsimd.tensor_tensor(out=gacc1[g][:], in0=gacc1[g][:],
                                        in1=mn[:], op=OP.add)
                yield
                if sfx == "a":
                    st["m_gat"] = keep
                    st["hT"] = hT_pool
                else:
                    st["m_gc"] = keep
                    st["zT"] = hT_pool
                    st["z_bf"] = h_pool

            def chain(*gens):
                for gg in gens:
                    yield from gg

            def drive(*streams):
                """round-robin the op streams until exhausted."""
                act = [iter(s) for s in streams if s is not None]
                while act:
                    for s in list(act):
                        try:
                            next(s)
                        except StopIteration:
                            act.remove(s)

            def issue_loads(g):
                xbf = stp.tile([P, NP_], BF16, tag="xbf")
                xbf3 = xbf[:].rearrange("p (c d) -> p c d", d=D)
                x_in3 = x_d[g].rearrange("(c p) d -> p c d", p=P)
                nc.sync.dma_start(out=xbf3[:, :, :], in_=x_in3[:, :, :])
                xT = stp.tile([P, NP_], BF16, tag="xT")
                nc.sync.dma_start(out=xT[:], in_=xT_d[g][:, :])
                cnt_t = cbp.tile([P, NT, NP_], BF16, tag="cnt")
                for q in range(4):
                    nc.sync.dma_start(
                        out=cnt_t[:, 2 * q:2 * q + 2, :],
                        in_=cnt_d[g].rearrange("(t p) v -> p t v", p=P)[
                            :, 2 * q:2 * q + 2, :])
                return dict(g=g, cnt=cnt_t[:].rearrange("p t v -> p (t v)"),
                            hT=xT, m_gat=m0, z_bf=xbf, zT=xT, m_gc=m0)

            def stage_gen(st, idx):
                """graph stage idx in 0..5: even = layers, odd = topks.
                Returns a list of independent op streams."""
                li = idx // 2
                if idx % 2 == 0:
                    return [gat_layer(st, li), gc_layer(st, li)]
                return [topk_readout(st, li, "a"),
                        topk_readout(st, li, "b")]

            # sliding schedule: graph g starts at step 6*(g//2) + (g%2);
            # every step pairs one graph's layer stage with the other's topk
            # stage (phase-offset op-level interleaving).
            start = {g: 6 * (g // 2) + (g % 2) for g in range(KG)}
            nsteps = max(start[g] + 6 for g in range(KG)) if KG else 0
            sts = {}
            for s in range(nsteps):
                for g in range(KG):
                    if start[g] - 2 == s or (start[g] <= 1 and s == 0 and g not in sts):
                        sts[g] = issue_loads(g)
                streams = []
                for g in range(KG):
                    idx = s - start[g]
                    if 0 <= idx < 6:
                        streams.extend(stage_gen(sts[g], idx))
                drive(*streams)

            if KDBG:
                dbg_src = None
                if _dbg_hook:
                    dbg_src = _dbg_hook[0]
                if dbg_src is not None:
                    dbf = vp.tile([P, NP_], F32, tag="dbf")
                    nc.vector.tensor_copy(out=dbf[:], in_=dbg_src[:])
                    nc.sync.dma_start(out=dbg_d[:], in_=dbf[:])
                else:
                    zz = vp.tile([P, NP_], F32, tag="zz")
                    nc.vector.memset(zz[:], 0.0)
                    nc.sync.dma_start(out=dbg_d[:], in_=zz[:])
            # ---- MLP over all graphs ----
            t1_ps = psT.tile([P, NT], F32, tag="pt")
            for g in range(G):
                nc.tensor.matmul(t1_ps[:, g:g + 1], Wl1a[:], gacc0[g][:],
                                 start=True, stop=False)
                nc.tensor.matmul(t1_ps[:, g:g + 1], Wl1b[:], gacc1[g][:],
                                 start=False, stop=True)
            t1 = vp.tile([P, G], F32, tag="t1")
            nc.vector.tensor_scalar(out=t1[:], in0=t1_ps[:, 0:G], scalar1=bl1[:, 0:1],
                                    scalar2=0.0, op0=OP.add, op1=OP.max)
            t2_ps = psT.tile([64, NT], F32, tag="pt")
            nc.tensor.matmul(t2_ps[:, 0:G], Wl2[:], t1[:], start=True, stop=True)
            t2p = vp.tile([64, G], F32, tag="t2p")
            nc.vector.tensor_scalar(out=t2p[:], in0=t2_ps[:, 0:G], scalar1=bl2[:, 0:1],
                                    scalar2=None, op0=OP.add)
            t2 = vp.tile([64, G], F32, tag="t2")
            nc.scalar.activation(out=t2[:], in_=t2p[:], func=AF.Prelu, alpha=0.01)
            t3_ps = psT.tile([C, 16], F32, tag="pt")
            nc.tensor.matmul(t3_ps[:, 0:G], Wl3[:], t2[:], start=True, stop=True)
            lg_cm = vp.tile([C, G], F32, tag="lgcm")
            nc.vector.tensor_scalar(out=lg_cm[:], in0=t3_ps[:, 0:G], scalar1=bl3[:, 0:1],
                                    scalar2=None, op0=OP.add)
            # transpose -> [G, C]
            lg_ps = psT.tile([G, 16], F32, tag="pt")
            nc.tensor.matmul(lg_ps[:, 0:C], lg_cm[:], ident[0:C, 0:C],
                             is_transpose=True)
            lg = vp.tile([G, C], F32, tag="lg")
            nc.vector.tensor_copy(out=lg[:], in_=lg_ps[:, 0:C])
            # log-sum-exp (logits are O(1))
            ex = vp.tile([G, C], F32, tag="ex")
            nc.scalar.activation(out=ex[:], in_=lg[:], func=AF.Exp)
            S = vp.tile([G, 1], F32, tag="S")
            nc.vector.tensor_reduce(out=S[:], in_=ex[:], axis=AX.X, op=OP.add)
            # ln(S) via Newton: y += S*exp(-y) - 1
            y = vp.tile([G, 1], F32, tag="y")
            nc.vector.memset(y[:], 2.3)
            for _ in range(6):
                eny = vp.tile([G, 1], F32, tag="eny")
                nc.scalar.activation(out=eny[:], in_=y[:], func=AF.Exp, scale=-1.0)
                nc.vector.tensor_tensor(out=eny[:], in0=eny[:], in1=S[:], op=OP.mult)
                nc.vector.tensor_scalar(out=eny[:], in0=eny[:], scalar1=1.0,
                                        scalar2=None, op0=OP.subtract)
                nc.vector.tensor_tensor(out=y[:], in0=y[:], in1=eny[:], op=OP.add)
            outt = vp.tile([G, C], F32, tag="outt")
            nc.vector.tensor_scalar(out=outt[:], in0=lg[:], scalar1=y[:, 0:1],
                                    scalar2=None, op0=OP.subtract)
            nc.sync.dma_start(out=out_d[:], in_=outt[:])

    nc.compile()
    return nc


# ----------------------------------------------------------------------------
# host side
# ----------------------------------------------------------------------------

def _prep_in_maps(inputs):
    import ml_dtypes
    BF = ml_dtypes.bfloat16
    x = np.ascontiguousarray(np.asarray(inputs["x"], np.float32))
    ei = np.asarray(inputs["edge_index"]).astype(np.int64)
    src, dst = ei[0], ei[1]
    gid = src // NPG
    sl, dl = src % NPG, dst % NPG

    cnt = np.zeros((B, NP_, NP_), np.int8)
    np.add.at(cnt, (gid, sl, dl), 1)
    cnt = cnt.astype(BF)

    x_pad = np.zeros((B, NP_, D), np.float32)
    x_pad[:, :NPG] = x.reshape(B, NPG, D)
    x_pad = x_pad.astype(BF)

    m0 = np.zeros((NP_,), np.float32)
    m0[:NPG] = 1.0
    m0_packed = np.ascontiguousarray(m0.reshape(NT, P).T)  # [P, NT]

    def col(v):
        return np.ascontiguousarray(np.asarray(v, np.float32).reshape(-1, 1))

    weights = {}
    for l in (1, 2, 3):
        weights[f"W_g{l}"] = np.ascontiguousarray(np.asarray(inputs[f"W_g{l}"], np.float32))
        Wg = np.asarray(inputs[f"W_g{l}"], np.float32)
        weights[f"asd_g{l}"] = np.ascontiguousarray(
            Wg @ np.stack([np.asarray(inputs[f"as_g{l}"], np.float32),
                           np.asarray(inputs[f"ad_g{l}"], np.float32)], axis=1))
        weights[f"b_g{l}"] = col(inputs[f"b_g{l}"])
        weights[f"Wr_c{l}"] = np.ascontiguousarray(np.asarray(inputs[f"Wr_c{l}"], np.float32))
        weights[f"br_c{l}"] = col(inputs[f"br_c{l}"])
        weights[f"Wo_c{l}"] = np.ascontiguousarray(np.asarray(inputs[f"Wo_c{l}"], np.float32))
    for n in ("w_p20", "w_p30", "w_p11", "w_p21", "w_p31"):
        w = np.asarray(inputs[n], np.float32)
        weights[n] = col(w / np.linalg.norm(w))
    weights["W_l1"] = np.ascontiguousarray(np.asarray(inputs["W_l1"], np.float32))
    weights["b_l1"] = col(inputs["b_l1"])
    weights["W_l2"] = np.ascontiguousarray(np.asarray(inputs["W_l2"], np.float32))
    weights["b_l2"] = col(inputs["b_l2"])
    weights["W_l3"] = np.ascontiguousarray(np.asarray(inputs["W_l3"], np.float32))
    weights["b_l3"] = col(inputs["b_l3"])

    in_maps = []
    for c in range(NCORES):
        lo = c * G
        hi = min(lo + G, B)
        xs = np.zeros((G, NP_, D), BF)
        cs = np.zeros((G, NP_, NP_), BF)
        if hi > lo:
            xs[:hi - lo] = x_pad[lo:hi]
            cs[:hi - lo] = cnt[lo:hi]
        xTs = np.ascontiguousarray(xs.transpose(0, 2, 1))
        im = {"x_sh": xs, "xT_sh": xTs, "cnt_sh": cs, "m0": m0_packed}
        im.update(weights)
        in_maps.append(im)
    return in_maps


def kernel(**inputs) -> np.ndarray:
    if "nc" not in _cache:
        _cache["nc"] = _build_program()
    nc = _cache["nc"]
    in_maps = _prep_in_maps(inputs)
    res = run_bass_kernel_spmd(nc, in_maps, list(range(NCORES)))
    out = np.zeros((B, C), np.float32)
    for c in range(NCORES):
        lo = c * G
        hi = min(lo + G, B)
        if hi > lo:
            out[lo:hi] = np.asarray(res.results[c]["out"])[:hi - lo]
    return out


# revision 110
# speedup vs baseline: 1.0002x; 1.0001x over previous
"""Trainium2 Bass kernel for nn_Net_60052232733176 (gnn_message_passing).

Strategy (graph-data parallel, 8 cores):
  - 50 graphs of 1000 nodes; core c handles graph slots [7c, 7c+7) (padded
    to 1024 nodes/graph).
  - Host side re-encodes each graph's edge list as a dense bf16 multiplicity
    matrix cnt[1024,1024] and ships x / x^T in bf16 (pure preprocessing).
  - On device everything is dense, mostly bf16: GAT attention logits are
    rank-1 (e_src[u]+e_dst[v]) built by PE broadcast-transposes; exp via
    two Act passes; masking by a DVE multiply with cnt; aggregation and
    per-dst softmax denominators via PE matmuls (den as one-shot per-column
    ones-matmuls per src tile — a PSUM bank only supports one OPEN
    accumulation group — summed on DVE). TopK pooling via dense rank
    counting (compare matrix + per-column ones-matmuls); readout max via
    tree reduce, mean via PE ones-matmuls; final MLP batched over graphs
    with log-softmax via Newton iterations for ln.
  - Scheduling: engine queues are strictly in-order, so the per-graph
    stages are emitted as Python generators and a sliding-window scheduler
    op-level-interleaves four independent streams (one graph's GAT+GC
    layer stages with the other graph's two topk stages, phase-offset) so
    every engine queue alternates ready work from independent chains.
  - No gather/scatter on device at all.

Self-contained: hardcodes all shapes; no file reads.
"""
import os
import numpy as np

import concourse.bass as bass
import concourse.bacc as bacc
import concourse.mybir as mybir
import concourse.tile as tile
from concourse.bass_utils import run_bass_kernel_spmd
from concourse.masks import make_identity
from concourse import bass_isa

F32 = mybir.dt.float32
BF16 = mybir.dt.bfloat16
FP16 = mybir.dt.float16
AF = mybir.ActivationFunctionType
OP = mybir.AluOpType
AX = mybir.AxisListType

P = 128
B, NPG, D, C = 50, 1000, 128, 10
NP_ = 1024            # padded nodes per graph
NT = NP_ // P         # 8 node tiles
NCORES = 8
G = 7                 # graph slots per core
K1, K2, K3 = 800, 640, 512
BIGM = 100.0          # dead-node fold added to e_src before exp
BIGS = 1.0e30         # dead-node fold for topk scores / readout max

_cache = {}


# ----------------------------------------------------------------------------
# device program
# ----------------------------------------------------------------------------

def _build_program():
    KG = int(os.environ.get("K_GRAPHS", G))
    KDBG = os.environ.get("K_DBG", "0") == "1"
    nc = bacc.Bacc(None, target_bir_lowering=False)

    # ---- DRAM tensors ----
    x_d = nc.dram_tensor("x_sh", [G, NP_, D], BF16, kind="ExternalInput")
    xT_d = nc.dram_tensor("xT_sh", [G, D, NP_], BF16, kind="ExternalInput")
    cnt_d = nc.dram_tensor("cnt_sh", [G, NP_, NP_], BF16, kind="ExternalInput")
    m0_d = nc.dram_tensor("m0", [P, NT], F32, kind="ExternalInput")

    def wparam(name, shape):
        return nc.dram_tensor(name, shape, F32, kind="ExternalInput")

    Wg_d = [wparam(f"W_g{l}", [D, D]) for l in (1, 2, 3)]
    asd_d = [wparam(f"asd_g{l}", [D, 2]) for l in (1, 2, 3)]
    bg_d = [wparam(f"b_g{l}", [D, 1]) for l in (1, 2, 3)]
    Wr_d = [wparam(f"Wr_c{l}", [D, D]) for l in (1, 2, 3)]
    br_d = [wparam(f"br_c{l}", [D, 1]) for l in (1, 2, 3)]
    Wo_d = [wparam(f"Wo_c{l}", [D, D]) for l in (1, 2, 3)]
    wp_d = {n: wparam(n, [D, 1]) for n in ("w_p20", "w_p30", "w_p11", "w_p21", "w_p31")}
    Wl1_d = wparam("W_l1", [2 * D, D])
    bl1_d = wparam("b_l1", [D, 1])
    Wl2_d = wparam("W_l2", [D, 64])
    bl2_d = wparam("b_l2", [64, 1])
    Wl3_d = wparam("W_l3", [64, C])
    bl3_d = wparam("b_l3", [C, 1])

    out_d = nc.dram_tensor("out", [G, C], F32, kind="ExternalOutput")
    dbg_d = nc.dram_tensor("dbg", [P, NP_], F32, kind="ExternalOutput") if KDBG else None

    with tile.TileContext(nc) as tc:
        import contextlib
        with contextlib.ExitStack() as ctx:
            cp = ctx.enter_context(tc.tile_pool(name="const", bufs=1))
            cbp = ctx.enter_context(tc.tile_pool(name="cntbf", bufs=3))
            Lp = ctx.enter_context(tc.tile_pool(name="Lp", bufs=6))
            Gp = ctx.enter_context(tc.tile_pool(name="Gp", bufs=6))
            stp = ctx.enter_context(tc.tile_pool(name="state", bufs=2))
            vp = ctx.enter_context(tc.tile_pool(name="vec", bufs=3))
            psA = ctx.enter_context(tc.tile_pool(name="psA", bufs=1, space="PSUM"))
            psT = ctx.enter_context(tc.tile_pool(name="psT", bufs=2, space="PSUM"))
            psR = ctx.enter_context(tc.tile_pool(name="psR", bufs=2, space="PSUM"))

            # ---- constants ----
            ident = cp.tile([P, P], F32, tag="ident")
            make_identity(nc, ident[:])
            ident_bf = cp.tile([P, P], BF16, tag="identbf")
            nc.vector.tensor_copy(out=ident_bf[:], in_=ident[:])
            ones_bf = cp.tile([P, 1], BF16, tag="onesbf")
            nc.vector.memset(ones_bf[:], 1.0)
            ones_f16 = cp.tile([P, 1], FP16, tag="onesf16")
            nc.vector.memset(ones_f16[:], 1.0)
            # PE warm: absorb gpsimd ident dep
            warm = psT.tile([P, P], F32, tag="pt")
            nc.tensor.transpose(out=warm[:], in_=ident[:], identity=ident[:])

            def load_w(dram, shape, tag, dtype=F32):
                t = cp.tile(shape, dtype, tag=tag)
                if dtype == F32:
                    nc.sync.dma_start(out=t[:], in_=dram[:])
                else:
                    tmp = cp.tile(shape, F32, tag=tag + "_st")
                    nc.sync.dma_start(out=tmp[:], in_=dram[:])
                    nc.vector.tensor_copy(out=t[:], in_=tmp[:])
                return t

            def col_to_rep(col_ap, out_tile_slice):
                """replicate a [128,1] column across partitions into [128,128]."""
                ps = psT.tile([P, P], F32, tag="pt")
                nc.tensor.transpose(out=ps[:], in_=col_ap.to_broadcast([P, P]),
                                    identity=ident[:])
                nc.vector.tensor_copy(out=out_tile_slice, in_=ps[:])

            Wg = [load_w(Wg_d[i], [D, D], f"Wg{i}", BF16) for i in range(3)]
            asd = [load_w(asd_d[i], [D, 2], f"asd{i}", BF16) for i in range(3)]
            bg_col = [load_w(bg_d[i], [D, 1], f"bg{i}") for i in range(3)]
            Wr_bf = [load_w(Wr_d[i], [D, D], f"Wr{i}", BF16) for i in range(3)]
            br_col = [load_w(br_d[i], [D, 1], f"br{i}") for i in range(3)]
            Wo_bf = [load_w(Wo_d[i], [D, D], f"Wo{i}", BF16) for i in range(3)]
            wp = {n: load_w(d, [D, 1], n) for n, d in wp_d.items()}
            Wl1a = cp.tile([D, D], F32, tag="Wl1a")
            nc.sync.dma_start(out=Wl1a[:], in_=Wl1_d[0:D, :])
            Wl1b = cp.tile([D, D], F32, tag="Wl1b")
            nc.sync.dma_start(out=Wl1b[:], in_=Wl1_d[D:2 * D, :])
            bl1 = load_w(bl1_d, [D, 1], "bl1")
            Wl2 = load_w(Wl2_d, [D, 64], "Wl2")
            bl2 = load_w(bl2_d, [64, 1], "bl2")
            Wl3 = load_w(Wl3_d, [64, C], "Wl3")
            bl3 = load_w(bl3_d, [C, 1], "bl3")
            m0 = load_w(m0_d, [P, NT], "m0")

            bg_rep = []
            for i in range(3):
                t = cp.tile([P, P], F32, tag=f"bgrep{i}")
                col_to_rep(bg_col[i][:, 0:1], t[:])
                bg_rep.append(t)
            wrep = {}
            for n in wp:
                tf = cp.tile([P, P], F32, tag=f"repf_{n}")
                col_to_rep(wp[n][:, 0:1], tf[:])
                t = cp.tile([P, P], BF16, tag=f"rep_{n}")
                nc.vector.tensor_copy(out=t[:], in_=tf[:])
                wrep[n] = t

            # per-graph readout accumulators
            gacc0 = []
            gacc1 = []
            for g in range(G):
                ga = cp.tile([P, 1], F32, tag=f"gacc0_{g}")
                gb = cp.tile([P, 1], F32, tag=f"gacc1_{g}")
                gacc0.append(ga)
                gacc1.append(gb)
            for g in range(G):
                nc.vector.memset(gacc0[g][:], 0.0)
                nc.vector.memset(gacc1[g][:], 0.0)

            CH = [slice(c * P, (c + 1) * P) for c in range(NT)]
            _dbg_hook = []
            pools_gat = [("w_p20", K1), ("w_p20", K2), ("w_p30", K3)]
            pools_gc = [("w_p11", K1), ("w_p21", K2), ("w_p31", K3)]

            def transpose_pack_bf(src_f32, dst_bf):
                """transpose [128,1024] f32 (node-major) -> [128,1024] bf16
                (feat-major) via two [128,512] PSUM half-passes."""
                for h in range(2):
                    pw = psT.tile([P, 512], F32, tag="pt")
                    for c in range(4):
                        nc.tensor.matmul(pw[:, c * P:(c + 1) * P],
                                         src_f32[:, CH[4 * h + c]], ident[:],
                                         is_transpose=True)
                    nc.scalar.activation(out=dst_bf[:, h * 512:(h + 1) * 512],
                                         in_=pw[:], func=AF.Copy)

            def chunk_tree_reduce(src, col_out, op):
                """src [128, 1024] node-major; col_out [128,1] = reduce."""
                t1 = vp.tile([P, 512], BF16, tag="rt1")
                nc.vector.tensor_tensor(out=t1[:], in0=src[:, :512], in1=src[:, 512:],
                                        op=op)
                nc.vector.tensor_tensor(out=t1[:, :256], in0=t1[:, :256],
                                        in1=t1[:, 256:], op=op)
                nc.vector.tensor_tensor(out=t1[:, :128], in0=t1[:, :128],
                                        in1=t1[:, 128:256], op=op)
                ps = psT.tile([P, P], BF16, tag="pt")
                nc.tensor.matmul(ps[:], t1[:, :128], ident_bf[:], is_transpose=True)
                nc.vector.tensor_reduce(out=col_out, in_=ps[:], axis=AX.X, op=op)

            def gat_layer(st, li, sfx="a"):
                """hT: feat-major bf16 [128,1024]. Sets st["ha"] (node-major
                [128,1024] f32, pre-pool). Generator: yields between op groups."""
                cnt_bf, hT, m_cur = st["cnt"], st["hT"], st["m_gat"]
                # hW node-major bf16 via two PSUM halves
                hW_bf = stp.tile([P, NP_], BF16, tag="hW" + sfx)
                for h in range(2):
                    pw = psT.tile([P, 512], F32, tag="pt")
                    for c in range(4):
                        nc.tensor.matmul(pw[:, c * P:(c + 1) * P],
                                         hT[:, CH[4 * h + c]], Wg[li][:],
                                         start=True, stop=True)
                    nc.scalar.activation(out=hW_bf[:, h * 512:(h + 1) * 512],
                                         in_=pw[:], func=AF.Copy)
                    yield
                # e vectors: [es | ed] per chunk (copies on DVE: keeps the
                # es2 -> Prelu-bias chain off the Act queue)
                est = vp.tile([P, 2 * NT], F32, tag="est" + sfx)
                for c in range(NT):
                    pe = psT.tile([P, 2], F32, tag="pt")
                    nc.tensor.matmul(pe[:], hT[:, CH[c]], asd[li][:],
                                     start=True, stop=True)
                    nc.vector.tensor_copy(out=est[:, 2 * c:2 * c + 2], in_=pe[:])
                    if c % 4 == 3:
                        yield
                est3 = est[:].rearrange("p (c two) -> p c two", two=2)
                est_e = est3[:, :, 0:1].rearrange("p c one -> p (c one)")
                est_o = est3[:, :, 1:2].rearrange("p c one -> p (c one)")
                # es2 = es + (m-1)*BIGM
                mf = vp.tile([P, NT], F32, tag="mf" + sfx)
                nc.vector.tensor_scalar(out=mf[:], in0=m_cur[:], scalar1=1.0,
                                        scalar2=BIGM, op0=OP.subtract, op1=OP.mult)
                es2 = vp.tile([P, NT], F32, tag="es2" + sfx)
                nc.vector.tensor_tensor(out=es2[:], in0=est_e, in1=mf[:], op=OP.add)
                yield
                # ed_rep [128, 1024] bf16 via two halves
                ed_rep = stp.tile([P, NP_], BF16, tag="edr" + sfx)
                for h in range(2):
                    pw = psT.tile([P, 512], F32, tag="pt")
                    for c in range(4):
                        cc = 4 * h + c
                        nc.tensor.transpose(
                            out=pw[:, c * P:(c + 1) * P],
                            in_=est[:, 2 * cc + 1:2 * cc + 2].to_broadcast([P, P]),
                            identity=ident[:])
                    nc.scalar.activation(out=ed_rep[:, h * 512:(h + 1) * 512],
                                         in_=pw[:], func=AF.Copy)
                    yield
                # self terms (early: off the critical tail)
                lself = vp.tile([P, NT], F32, tag="ls" + sfx)
                nc.vector.tensor_tensor(out=lself[:], in0=es2[:], in1=est_o,
                                        op=OP.add)
                nc.scalar.activation(out=lself[:], in_=lself[:], func=AF.Prelu,
                                     alpha=0.2)
                nc.scalar.activation(out=lself[:], in_=lself[:], func=AF.Exp)
                yield
                # L tiles: lrelu(ed_rep + es_u) -> exp -> * cnt; agg + den
                # matmuls (den: one-shot per-column matmuls per t — a PSUM bank
                # can only hold one OPEN accumulation group — summed on DVE)
                agg_ps = psA.tile([P, NP_], F32, tag="agA")
                den_sb = vp.tile([P, NT], F32, tag="dn" + sfx)
                for t in range(NT):
                    Lt = Lp.tile([P, NP_], BF16, tag="L")
                    nc.scalar.activation(out=Lt[:], in_=ed_rep[:], func=AF.Prelu,
                                         alpha=0.2, bias=es2[:, t:t + 1])
                    nc.scalar.activation(out=Lt[:], in_=Lt[:], func=AF.Exp)
                    yield
                    nc.vector.tensor_tensor(out=Lt[:], in0=Lt[:],
                                            in1=cnt_bf[:, t * NP_:(t + 1) * NP_],
                                            op=OP.mult)
                    for h in range(2):
                        nc.tensor.matmul(
                            agg_ps[:, h * 512:(h + 1) * 512],
                            hW_bf[:, CH[t]], Lt[:, h * 512:(h + 1) * 512],
                            start=(t == 0), stop=(t == NT - 1))
                    psd = psR.tile([P, NT], F32, tag="pc")
                    for c in range(NT):
                        nc.tensor.matmul(
                            psd[:, c:c + 1], Lt[:, CH[c]], ones_bf[:],
                            start=True, stop=True)
                    if t == 0:
                        nc.vector.tensor_copy(out=den_sb[:], in_=psd[:])
                    else:
                        nc.vector.tensor_tensor(out=den_sb[:], in0=den_sb[:],
                                                in1=psd[:], op=OP.add)
                    yield
                dtot = vp.tile([P, NT], F32, tag="dt" + sfx)
                nc.vector.tensor_tensor(out=dtot[:], in0=den_sb[:], in1=lself[:],
                                        op=OP.add)
                if os.environ.get("K_DBG_PICK", "") == f"stats{sfx}{li}":
                    dbst = stp.tile([P, NP_], F32, tag="dbst")
                    nc.vector.memset(dbst[:], 0.0)
                    nc.vector.tensor_copy(out=dbst[:, 0:NT], in_=dtot[:])
                    nc.vector.tensor_copy(out=dbst[:, NT:2 * NT], in_=es2[:])
                    nc.vector.tensor_copy(out=dbst[:, 2 * NT:3 * NT], in_=lself[:])
                    nc.vector.tensor_copy(out=dbst[:, 3 * NT:4 * NT], in_=est_o)
                    _dbg_hook.append(dbst)
                rd = vp.tile([P, NT], F32, tag="rd" + sfx)
                nc.vector.reciprocal(out=rd[:], in_=dtot[:])
                csel = vp.tile([P, NT], F32, tag="cs" + sfx)
                nc.vector.tensor_tensor(out=csel[:], in0=lself[:], in1=rd[:],
                                        op=OP.mult)
                yield
                # evacuate agg (feat-major) to SBUF for transpose
                outT_sb = stp.tile([P, NP_], F32, tag="oT" + sfx)
                for h in range(2):
                    nc.scalar.activation(out=outT_sb[:, h * 512:(h + 1) * 512],
                                         in_=agg_ps[:, h * 512:(h + 1) * 512],
                                         func=AF.Copy)
                    yield
                # finalize node-major: relu((aggT*rd + hW*csel + b) * m)
                h_next = stp.tile([P, NP_], BF16, tag="hn" + sfx)
                for h in range(2):
                    pw = psT.tile([P, 512], F32, tag="pt")
                    for c in range(4):
                        nc.tensor.matmul(pw[:, c * P:(c + 1) * P],
                                         outT_sb[:, CH[4 * h + c]], ident[:],
                                         is_transpose=True)
                    for c in range(4):
                        cc = 4 * h + c
                        f1 = vp.tile([P, P], F32, tag="f1" + sfx)
                        nc.vector.scalar_tensor_tensor(
                            out=f1[:], in0=hW_bf[:, CH[cc]],
                            scalar=csel[:, cc:cc + 1], in1=bg_rep[li][:],
                            op0=OP.mult, op1=OP.add)
                        f2 = vp.tile([P, P], F32, tag="f2" + sfx)
                        nc.vector.scalar_tensor_tensor(
                            out=f2[:], in0=pw[:, c * P:(c + 1) * P],
                            scalar=rd[:, cc:cc + 1], in1=f1[:],
                            op0=OP.mult, op1=OP.add)
                        nc.vector.tensor_scalar(
                            out=h_next[:, CH[cc]], in0=f2[:],
                            scalar1=m_cur[:, cc:cc + 1], scalar2=0.0,
                            op0=OP.mult, op1=OP.max)
                    yield
                st["ha"] = h_next
                if os.environ.get("K_DBG_PICK", "a2") == f"a{li}":
                    _dbg_hook.append(h_next)

            def gc_layer(st, li, sfx="b"):
                """GraphConv: relu((lin_rel(sum_src z) + lin_root(z)) * m).
                Sets st["hb"]. Generator."""
                cnt_bf, z_bf, zT, m_cur = st["cnt"], st["z_bf"], st["zT"], st["m_gc"]
                agg_ps = psA.tile([P, NP_], F32, tag="agB")
                for t in range(NT):
                    for h in range(2):
                        nc.tensor.matmul(
                            agg_ps[:, h * 512:(h + 1) * 512],
                            z_bf[:, CH[t]],
                            cnt_bf[:, t * NP_ + h * 512: t * NP_ + (h + 1) * 512],
                            start=(t == 0), stop=(t == NT - 1))
                    if t % 2 == 1:
                        yield
                aggT_bf = stp.tile([P, NP_], BF16, tag="agb" + sfx)
                for h in range(2):
                    nc.scalar.activation(out=aggT_bf[:, h * 512:(h + 1) * 512],
                                         in_=agg_ps[:, h * 512:(h + 1) * 512],
                                         func=AF.Copy)
                    yield
                outT_ps = psA.tile([P, NP_], F32, tag="agB")
                for h in range(2):
                    sl = slice(h * 512, (h + 1) * 512)
                    nc.tensor.matmul(outT_ps[:, sl], Wr_bf[li][:], aggT_bf[:, sl],
                                     start=True, stop=False)
                    nc.tensor.matmul(outT_ps[:, sl], Wo_bf[li][:], zT[:, sl],
                                     start=False, stop=True)
                    yield
                # + bias (per-feature = per-partition in feat-major)
                outT_sb = stp.tile([P, NP_], F32, tag="oT" + sfx)
                for h in range(2):
                    nc.scalar.activation(out=outT_sb[:, h * 512:(h + 1) * 512],
                                         in_=outT_ps[:, h * 512:(h + 1) * 512],
                                         func=AF.Identity, bias=br_col[li][:, 0:1])
                    yield
                h_next = stp.tile([P, NP_], BF16, tag="hn" + sfx)
                for h in range(2):
                    pw = psT.tile([P, 512], F32, tag="pt")
                    for c in range(4):
                        nc.tensor.matmul(pw[:, c * P:(c + 1) * P],
                                         outT_sb[:, CH[4 * h + c]], ident[:],
                                         is_transpose=True)
                    for c in range(4):
                        cc = 4 * h + c
                        nc.vector.tensor_scalar(
                            out=h_next[:, CH[cc]], in0=pw[:, c * P:(c + 1) * P],
                            scalar1=m_cur[:, cc:cc + 1], scalar2=0.0,
                            op0=OP.mult, op1=OP.max)
                    yield
                st["hb"] = h_next
                if os.environ.get("K_DBG_PICK", "a2") == f"b{li}":
                    _dbg_hook.append(h_next)

            def topk_readout(st, li, sfx):
                """pool branch output by top-k of scores; update st state;
                accumulate readout into gacc. Generator."""
                g = st["g"]
                if sfx == "a":
                    h_next, m_cur = st["ha"], st["m_gat"]
                    wn, k = pools_gat[li]
                else:
                    h_next, m_cur = st["hb"], st["m_gc"]
                    wn, k = pools_gc[li]
                wrep_t = wrep[wn]
                need_hT = li < 2
                # mask fold first: depends only on m_cur (ready at entry)
                mf = vp.tile([P, NT], F32, tag="mfs" + sfx)
                nc.vector.tensor_scalar(out=mf[:], in0=m_cur[:], scalar1=1.0,
                                        scalar2=BIGS, op0=OP.subtract, op1=OP.mult)
                # scores via fused mult + free-dim accumulate
                s = vp.tile([P, NT], F32, tag="s" + sfx)
                jnk = vp.tile([P, P], BF16, tag="jk" + sfx)
                for c in range(NT):
                    nc.vector.scalar_tensor_tensor(
                        out=jnk[:], in0=h_next[:, CH[c]], scalar=1.0,
                        in1=wrep_t[:], op0=OP.mult, op1=OP.mult,
                        accum_out=s[:, c:c + 1])
                    if c % 2 == 1:
                        yield
                # masked scores s' = s + (m-1)*BIGS
                sm = vp.tile([P, NT], F32, tag="sm" + sfx)
                nc.vector.tensor_tensor(out=sm[:], in0=s[:], in1=mf[:], op=OP.add)
                yield
                # s_rep [128, 1024] f32 via two halves
                srep = stp.tile([P, NP_], F32, tag="sr" + sfx, bufs=3)
                for h in range(2):
                    pw = psT.tile([P, 512], F32, tag="pt")
                    for c in range(4):
                        cc = 4 * h + c
                        nc.tensor.transpose(
                            out=pw[:, c * P:(c + 1) * P],
                            in_=sm[:, cc:cc + 1].to_broadcast([P, P]),
                            identity=ident[:])
                    nc.scalar.activation(out=srep[:, h * 512:(h + 1) * 512],
                                         in_=pw[:], func=AF.Copy)
                    yield
                # G[j, i] = s'[i] < s'[j]; rank[i] = sum_j G[j, i] (column form,
                # one-shot per-column matmuls per t, summed on DVE)
                th = vp.tile([P, NT], F32, tag="th" + sfx)
                nc.scalar.activation(out=th[:], in_=s[:], func=AF.Tanh)
                rank_sb = vp.tile([P, NT], F32, tag="rk" + sfx)
                for t in range(NT):
                    Gt = Gp.tile([P, NP_], FP16, tag="G")
                    eng = nc.vector if t % 2 == 0 else nc.gpsimd
                    eng.tensor_scalar(
                        out=Gt[:], in0=srep[:],
                        scalar1=sm[:, t:t + 1], scalar2=None, op0=OP.is_lt)
                    psr = psR.tile([P, NT], F32, tag="pc")
                    for c in range(NT):
                        nc.tensor.matmul(
                            psr[:, c:c + 1], Gt[:, CH[c]], ones_f16[:],
                            start=True, stop=True)
                    if t == 0:
                        nc.vector.tensor_copy(out=rank_sb[:], in_=psr[:])
                    else:
                        nc.vector.tensor_tensor(out=rank_sb[:], in0=rank_sb[:],
                                                in1=psr[:], op=OP.add)
                    yield
                keep = vp.tile([P, NT], F32, tag="kp" + sfx)
                nc.vector.tensor_scalar(out=keep[:], in0=rank_sb[:],
                                        scalar1=float(k),
                                        scalar2=None, op0=OP.is_lt)
                # pool scale = tanh(s) * keep ; kf = (keep-1)*BIGS
                pool = vp.tile([P, NT], F32, tag="pl" + sfx)
                nc.vector.tensor_tensor(out=pool[:], in0=th[:], in1=keep[:],
                                        op=OP.mult)
                kf = vp.tile([P, NT], F32, tag="kf" + sfx)
                nc.vector.tensor_scalar(out=kf[:], in0=keep[:], scalar1=1.0,
                                        scalar2=BIGS, op0=OP.subtract, op1=OP.mult)
                yield
                h_pool = stp.tile([P, NP_], BF16, tag="hp" + sfx)
                hm = stp.tile([P, NP_], BF16, tag="sr" + sfx, bufs=3)  # srep dead here
                for c in range(NT):
                    nc.vector.tensor_scalar(out=h_pool[:, CH[c]],
                                            in0=h_next[:, CH[c]],
                                            scalar1=pool[:, c:c + 1], scalar2=None,
                                            op0=OP.mult)
                    if c % 2 == 1:
                        yield
                for c in range(NT):
                    heng = nc.vector if c % 4 == 0 else nc.gpsimd
                    heng.tensor_scalar(out=hm[:, CH[c]], in0=h_next[:, CH[c]],
                                       scalar1=pool[:, c:c + 1],
                                       scalar2=kf[:, c:c + 1],
                                       op0=OP.mult, op1=OP.add)
                    if c % 2 == 1:
                        yield
                # hT for the next layer FIRST: it gates the next gat/gc
                # stage, while the readout below feeds only the final MLP
                hT_pool = None
                if need_hT:
                    hT_pool = stp.tile([P, NP_], BF16, tag="hT" + sfx)
                    for h in range(2):
                        pw = psT.tile([P, 512], BF16, tag="pt")
                        for c in range(4):
                            nc.tensor.matmul(pw[:, c * P:(c + 1) * P],
                                             h_pool[:, CH[4 * h + c]], ident_bf[:],
                                             is_transpose=True)
                        nc.scalar.activation(out=hT_pool[:, h * 512:(h + 1) * 512],
                                             in_=pw[:], func=AF.Copy)
                        yield
                # readout: masked max + mean/k
                mx = vp.tile([P, 1], F32, tag="mx" + sfx)
                chunk_tree_reduce(hm[:], mx[:], OP.max)
                nc.gpsimd.tensor_tensor(out=gacc0[g][:], in0=gacc0[g][:],
                                        in1=mx[:], op=OP.add)
                yield
                psm = psT.tile([P, 2], F32, tag="pt")
                for c in range(NT):
                    nc.tensor.matmul(psm[:, 0:1], h_pool[:, CH[c]], ones_bf[:],
                                     start=(c == 0), stop=(c == NT - 1))
                mn = vp.tile([P, 1], F32, tag="mn" + sfx)
                nc.vector.tensor_scalar(out=mn[:], in0=psm[:, 0:1], scalar1=1.0 / k,
                                        scalar2=None, op0=OP.mult)
                nc.gpsimd.tensor_tensor(out=gacc1[g][:], in0=gacc1[g][:],
                                        in1=mn[:], op=OP.add)
                yield
                if sfx == "a":
                    st["m_gat"] = keep
                    st["hT"] = hT_pool
                else:
                    st["m_gc"] = keep
                    st["zT"] = hT_pool
                    st["z_bf"] = h_pool

            def chain(*gens):
                for gg in gens:
                    yield from gg

            def drive(*streams):
                """round-robin the op streams until exhausted."""
                act = [iter(s) for s in streams if s is not None]
                while act:
                    for s in list(act):
                        try:
                            next(s)
                        except StopIteration:
                            act.remove(s)

            def issue_loads(g):
                xbf = stp.tile([P, NP_], BF16, tag="xbf")
                xbf3 = xbf[:].rearrange("p (c d) -> p c d", d=D)
                x_in3 = x_d[g].rearrange("(c p) d -> p c d", p=P)
                nc.sync.dma_start(out=xbf3[:, :, :], in_=x_in3[:, :, :])
                xT = stp.tile([P, NP_], BF16, tag="xT")
                nc.sync.dma_start(out=xT[:], in_=xT_d[g][:, :])
                cnt_t = cbp.tile([P, NT, NP_], BF16, tag="cnt")
                for q in range(4):
                    nc.sync.dma_start(
                        out=cnt_t[:, 2 * q:2 * q + 2, :],
                        in_=cnt_d[g].rearrange("(t p) v -> p t v", p=P)[
                            :, 2 * q:2 * q + 2, :])
                return dict(g=g, cnt=cnt_t[:].rearrange("p t v -> p (t v)"),
                            hT=xT, m_gat=m0, z_bf=xbf, zT=xT, m_gc=m0)

            def stage_gen(st, idx):
                """graph stage idx in 0..5: even = layers, odd = topks.
                Returns a list of independent op streams."""
                li = idx // 2
                if idx % 2 == 0:
                    return [gat_layer(st, li), gc_layer(st, li)]
                return [topk_readout(st, li, "a"),
                        topk_readout(st, li, "b")]

            # sliding schedule: graph g starts at step 6*(g//2) + (g%2);
            # every step pairs one graph's layer stage with the other's topk
            # stage (phase-offset op-level interleaving).
            start = {g: 6 * (g // 2) + (g % 2) for g in range(KG)}
            nsteps = max(start[g] + 6 for g in range(KG)) if KG else 0
            sts = {}
            for s in range(nsteps):
                for g in range(KG):
                    if start[g] - 2 == s or (start[g] <= 1 and s == 0 and g not in sts):
                        sts[g] = issue_loads(g)
                streams = []
                for g in range(KG):
                    idx = s - start[g]
                    if 0 <= idx < 6:
                        streams.extend(stage_gen(sts[g], idx))
                drive(*streams)

            if KDBG:
                dbg_src = None
                if _dbg_hook:
                    dbg_src = _dbg_hook[0]
                if dbg_src is not None:
                    dbf = vp.tile([P, NP_], F32, tag="dbf")
                    nc.vector.tensor_copy(out=dbf[:], in_=dbg_src[:])
                    nc.sync.dma_start(out=dbg_d[:], in_=dbf[:])
                else:
                    zz = vp.tile([P, NP_], F32, tag="zz")
                    nc.vector.memset(zz[:], 0.0)
                    nc.sync.dma_start(out=dbg_d[:], in_=zz[:])
            # ---- MLP over all graphs ----
            t1_ps = psT.tile([P, NT], F32, tag="pt")
            for g in range(G):
                nc.tensor.matmul(t1_ps[:, g:g + 1], Wl1a[:], gacc0[g][:],
                                 start=True, stop=False)
                nc.tensor.matmul(t1_ps[:, g:g + 1], Wl1b[:], gacc1[g][:],
                                 start=False, stop=True)
            t1 = vp.tile([P, G], F32, tag="t1")
            nc.vector.tensor_scalar(out=t1[:], in0=t1_ps[:, 0:G], scalar1=bl1[:, 0:1],
                                    scalar2=0.0, op0=OP.add, op1=OP.max)
            t2_ps = psT.tile([64, NT], F32, tag="pt")
            nc.tensor.matmul(t2_ps[:, 0:G], Wl2[:], t1[:], start=True, stop=True)
            t2p = vp.tile([64, G], F32, tag="t2p")
            nc.vector.tensor_scalar(out=t2p[:], in0=t2_ps[:, 0:G], scalar1=bl2[:, 0:1],
                                    scalar2=None, op0=OP.add)
            t2 = vp.tile([64, G], F32, tag="t2")
            nc.scalar.activation(out=t2[:], in_=t2p[:], func=AF.Prelu, alpha=0.01)
            t3_ps = psT.tile([C, 16], F32, tag="pt")
            nc.tensor.matmul(t3_ps[:, 0:G], Wl3[:], t2[:], start=True, stop=True)
            lg_cm = vp.tile([C, G], F32, tag="lgcm")
            nc.vector.tensor_scalar(out=lg_cm[:], in0=t3_ps[:, 0:G], scalar1=bl3[:, 0:1],
                                    scalar2=None, op0=OP.add)
            # transpose -> [G, C]
            lg_ps = psT.tile([G, 16], F32, tag="pt")
            nc.tensor.matmul(lg_ps[:, 0:C], lg_cm[:], ident[0:C, 0:C],
                             is_transpose=True)
            lg = vp.tile([G, C], F32, tag="lg")
            nc.vector.tensor_copy(out=lg[:], in_=lg_ps[:, 0:C])
            # log-sum-exp (logits are O(1))
            ex = vp.tile([G, C], F32, tag="ex")
            nc.scalar.activation(out=ex[:], in_=lg[:], func=AF.Exp)
            S = vp.tile([G, 1], F32, tag="S")
            nc.vector.tensor_reduce(out=S[:], in_=ex[:], axis=AX.X, op=OP.add)
            # ln(S) via Newton: y += S*exp(-y) - 1
            y = vp.tile([G, 1], F32, tag="y")
            nc.vector.memset(y[:], 2.3)
            for _ in range(6):
                eny = vp.tile([G, 1], F32, tag="eny")
                nc.scalar.activation(out=eny[:], in_=y[:], func=AF.Exp, scale=-1.0)
                nc.vector.tensor_tensor(out=eny[:], in0=eny[:], in1=S[:], op=OP.mult)
                nc.vector.tensor_scalar(out=eny[:], in0=eny[:], scalar1=1.0,
                                        scalar2=None, op0=OP.subtract)
                nc.vector.tensor_tensor(out=y[:], in0=y[:], in1=eny[:], op=OP.add)
            outt = vp.tile([G, C], F32, tag="outt")
            nc.vector.tensor_scalar(out=outt[:], in0=lg[:], scalar1=y[:, 0:1],
                                    scalar2=None, op0=OP.subtract)
            nc.sync.dma_start(out=out_d[:], in_=outt[:])

    nc.compile()
    return nc


# ----------------------------------------------------------------------------
# host side
# ----------------------------------------------------------------------------

def _prep_in_maps(inputs):
    import ml_dtypes
    BF = ml_dtypes.bfloat16
    x = np.ascontiguousarray(np.asarray(inputs["x"], np.float32))
    ei = np.asarray(inputs["edge_index"]).astype(np.int64)
    src, dst = ei[0], ei[1]
    gid = src // NPG
    sl, dl = src % NPG, dst % NPG

    cnt = np.zeros((B, NP_, NP_), np.int8)
    np.add.at(cnt, (gid, sl, dl), 1)
    cnt = cnt.astype(BF)

    x_pad = np.zeros((B, NP_, D), np.float32)
    x_pad[:, :NPG] = x.reshape(B, NPG, D)
    x_pad = x_pad.astype(BF)

    m0 = np.zeros((NP_,), np.float32)
    m0[:NPG] = 1.0
    m0_packed = np.ascontiguousarray(m0.reshape(NT, P).T)  # [P, NT]

    def col(v):
        return np.ascontiguousarray(np.asarray(v, np.float32).reshape(-1, 1))

    weights = {}
    for l in (1, 2, 3):
        weights[f"W_g{l}"] = np.ascontiguousarray(np.asarray(inputs[f"W_g{l}"], np.float32))
        Wg = np.asarray(inputs[f"W_g{l}"], np.float32)
        weights[f"asd_g{l}"] = np.ascontiguousarray(
            Wg @ np.stack([np.asarray(inputs[f"as_g{l}"], np.float32),
                           np.asarray(inputs[f"ad_g{l}"], np.float32)], axis=1))
        weights[f"b_g{l}"] = col(inputs[f"b_g{l}"])
        weights[f"Wr_c{l}"] = np.ascontiguousarray(np.asarray(inputs[f"Wr_c{l}"], np.float32))
        weights[f"br_c{l}"] = col(inputs[f"br_c{l}"])
        weights[f"Wo_c{l}"] = np.ascontiguousarray(np.asarray(inputs[f"Wo_c{l}"], np.float32))
    for n in ("w_p20", "w_p30", "w_p11", "w_p21", "w_p31"):
        w = np.asarray(inputs[n], np.float32)
        weights[n] = col(w / np.linalg.norm(w))
    weights["W_l1"] = np.ascontiguousarray(np.asarray(inputs["W_l1"], np.float32))
    weights["b_l1"] = col(inputs["b_l1"])
    weights["W_l2"] = np.ascontiguousarray(np.asarray(inputs["W_l2"], np.float32))
    weights["b_l2"] = col(inputs["b_l2"])
    weights["W_l3"] = np.ascontiguousarray(np.asarray(inputs["W_l3"], np.float32))
    weights["b_l3"] = col(inputs["b_l3"])

    in_maps = []
    for c in range(NCORES):
        lo = c * G
        hi = min(lo + G, B)
        xs = np.zeros((G, NP_, D), BF)
        cs = np.zeros((G, NP_, NP_), BF)
        if hi > lo:
            xs[:hi - lo] = x_pad[lo:hi]
            cs[:hi - lo] = cnt[lo:hi]
        xTs = np.ascontiguousarray(xs.transpose(0, 2, 1))
        im = {"x_sh": xs, "xT_sh": xTs, "cnt_sh": cs, "m0": m0_packed}
        im.update(weights)
        in_maps.append(im)
    return in_maps


def kernel(**inputs) -> np.ndarray:
    if "nc" not in _cache:
        _cache["nc"] = _build_program()
    nc = _cache["nc"]
    in_maps = _prep_in_maps(inputs)
    res = run_bass_kernel_spmd(nc, in_maps, list(range(NCORES)))
    out = np.zeros((B, C), np.float32)
    for c in range(NCORES):
        lo = c * G
        hi = min(lo + G, B)
        if hi > lo:
            out[lo:hi] = np.asarray(res.results[c]["out"])[:hi - lo]
    return out
